# revision 1
# baseline (speedup 1.0000x reference)
"""Trainium2 Bass kernel for nn_FLASH_40458591928592 (sparse_attention).

Sequence-sharded over 8 NeuronCores: 1024 tokens (= 4 groups of 256) per core.
All matmuls bf16 operands / fp32 PSUM accumulate. Residual +x in fp32.

Per-core device program:
  qk   = silu(x_s @ W_qk + b_qk)            feature-major  [128qk, 1024tok]
  v    = silu(x_0 @ W_h[:, :H] + b_h[:H])   token-major    [1024tok, 2048]
  gate = silu(x_s @ W_h[:, H:] + b_h[H:])   feature-major  [2048hc, 1024tok]
  Quadratic group attention with the causal depthwise conv folded in as a
  constant banded add to attnT plus a K=32 boundary matmul over the 32-token
  tail of the previous group (halo-computed for the first local group).
  lin_kv prefix scan: per-group kv accumulated in one PSUM bank with
  snapshots; per-core totals AllGathered per e-half (bf16); each core applies
  a 0/1 weighted sum (host-provided weights) for the exclusive prefix.
  out  = quadT + linT + convT               feature-major  [2048, 1024tok]
  y_s  = (gate_s*out)^T @ W_out + b_out + x_s  token-major, written fp32.

Matmuls that share a stationary operand are paired (one LDWEIGHTS feeds
2 matmuls into different PSUM banks) to hide weight-load latency.
"""

from contextlib import ExitStack

import numpy as np
import ml_dtypes

import concourse.tile as tile
from concourse import bacc, mybir
from concourse.bass_utils import run_bass_kernel_spmd
from concourse.masks import make_identity

BF = mybir.dt.bfloat16
F32 = mybir.dt.float32
bf16 = ml_dtypes.bfloat16

G = 256
DIM = 1024
HID = 2048
DQK = 128
NSEQ = 8192
NC = 8
T = NSEQ // NC        # 1024 tokens per core
NG = T // G           # 4 groups per core
KD = DIM // 128       # 8 k-tiles over dim
ET = HID // 128       # 16 e-tiles over hid
TT = T // 128         # 8 token tiles
EH = HID // 2         # 1024 cols per e-half

AF = mybir.ActivationFunctionType
ALU = mybir.AluOpType

DEBUG_DUMPS = False
WITH_VBIAS = True   # emit ones-row matmuls for b_h[:HID] (skipped when zero)
WITH_OBIAS = True   # emit ones-row matmuls for b_out (skipped when zero)


def _emit(tc, ap):
    nc = tc.nc
    with ExitStack() as ctx:
        consts = ctx.enter_context(tc.tile_pool(name="consts", bufs=1))
        p_xt = ctx.enter_context(tc.tile_pool(name="xt", bufs=4))
        p_qkT = ctx.enter_context(tc.tile_pool(name="qkT", bufs=1))
        p_lk = ctx.enter_context(tc.tile_pool(name="lk", bufs=1))
        p_w = ctx.enter_context(tc.tile_pool(name="w", bufs=1))      # Wv -> Wg
        p_sw = ctx.enter_context(tc.tile_pool(name="sw", bufs=1))    # S_full -> Wout
        p_big = ctx.enter_context(tc.tile_pool(name="big", bufs=1))  # v_h -> og
        p_tails = ctx.enter_context(tc.tile_pool(name="tails", bufs=1))
        p_so = ctx.enter_context(tc.tile_pool(name="so", bufs=2))
        p_sob = ctx.enter_context(tc.tile_pool(name="sob", bufs=1))
        p_tr = ctx.enter_context(tc.tile_pool(name="tr", bufs=4))
        p_a0 = ctx.enter_context(tc.tile_pool(name="a0", bufs=4))
        p_a1 = ctx.enter_context(tc.tile_pool(name="a1", bufs=4))
        p_outT = ctx.enter_context(tc.tile_pool(name="outT", bufs=1))
        p_y = ctx.enter_context(tc.tile_pool(name="ysb", bufs=2))
        p_xr = ctx.enter_context(tc.tile_pool(name="xr", bufs=2))
        ps1 = ctx.enter_context(tc.tile_pool(name="ps1", bufs=4, space="PSUM"))
        ps2 = ctx.enter_context(tc.tile_pool(name="ps2", bufs=2, space="PSUM"))
        pskv = ctx.enter_context(tc.tile_pool(name="pskv", bufs=2, space="PSUM"))

        # ---- first the DMAs that gate the first PE work ----
        # consts/weights go on the ACT HWDGE queue so xt streams don't queue
        # behind them on SP
        wqk_sb = consts.tile([128, KD, DQK], BF, tag="wqk")
        nc.scalar.dma_start(wqk_sb, ap["wqk"].rearrange("(kt p) q -> p kt q", p=128))

        def load_xt(s):
            halves = []
            for h in range(2):
                t = p_xt.tile([128, KD // 2, T], BF, tag="xt")
                # two DMAs per half so dependent matmuls start sooner
                for q in range(2):
                    r0 = h * 512 + q * 256
                    nc.sync.dma_start(
                        t[:, q * 2:(q + 1) * 2, :],
                        ap["xt"][s, r0:r0 + 256, :].rearrange(
                            "(kt p) t -> p kt t", p=128))
                halves.append(t)
            return halves

        def xt_k(halves, kt):
            return halves[kt // 4][:, kt % 4, :]

        bqk = consts.tile([128, 1], F32, tag="bqk")
        nc.scalar.dma_start(bqk, ap["bqk"])

        qkT = p_qkT.tile([128, 4, T], BF, tag="qkT")
        xt0 = None

        # prefetch Wv early (first big PE phase after qk needs it)
        wv_sb = p_w.tile([128, KD, HID], BF, tag="w")
        nc.scalar.dma_start(wv_sb, ap["wv"].rearrange("(kt p) e -> p kt e", p=128))

        # warm-up collective: absorbs the first-collective rank-skew /
        # setup latency (~40us) so the real AllGathers run at wire speed
        warm = consts.tile([128, 16], BF, tag="warm")
        nc.vector.memset(warm, 0.0)
        nc.sync.dma_start(ap["cc_warm_in"], warm)
        nc.gpsimd.collective_compute(
            "AllGather", ALU.bypass, replica_groups=[list(range(NC))],
            ins=[ap["cc_warm_in"]], outs=[ap["cc_warm_out"]])

        def qk_stream(s, halves):
            for ch in range(2):
                ps = ps1.tile([128, 512], F32, tag="ps1")
                for kt in range(KD):
                    nc.tensor.matmul(
                        ps, wqk_sb[:, kt, :],
                        xt_k(halves, kt)[:, ch * 512:(ch + 1) * 512],
                        start=(kt == 0), stop=(kt == KD - 1))
                nc.scalar.activation(qkT[:, s, ch * 512:(ch + 1) * 512], ps,
                                     AF.Silu, bias=bqk, scale=1.0)

        halves = load_xt(1)
        qk_stream(1, halves)

        # ---- remaining constants (DMA while PE chews on qk) ----
        ident = consts.tile([128, 128], BF, tag="ident")
        make_identity(nc, ident)
        ones_t = consts.tile([1, 1024], BF, tag="ones")
        nc.vector.memset(ones_t, 1.0)
        triu = consts.tile([128, 128], BF, tag="triu")
        nc.scalar.dma_start(triu, ap["triu"])
        bdiag = consts.tile([128, 128], BF, tag="bdiag")
        nc.scalar.dma_start(bdiag, ap["bdiag"])
        bcorn = consts.tile([128, 128], BF, tag="bcorn")
        nc.scalar.dma_start(bcorn, ap["bcorn"])
        bprev = consts.tile([32, 32], BF, tag="bprev")
        nc.scalar.dma_start(bprev, ap["bprev"])
        hmask = consts.tile([32, 1], F32, tag="hmask")
        nc.scalar.dma_start(hmask, ap["hmask"])
        wsumw = consts.tile([128, NC], F32, tag="wsumw")
        nc.scalar.dma_start(wsumw, ap["wsumw"])
        bgate = consts.tile([128, ET], F32, tag="bgate")
        nc.scalar.dma_start(bgate, ap["bgate"])
        wvb = consts.tile([1, HID], BF, tag="wvb")
        nc.scalar.dma_start(wvb, ap["wvb"])
        bout = consts.tile([1, DIM], BF, tag="bout")
        nc.scalar.dma_start(bout, ap["bout"])
        xh_sb = consts.tile([128, KD, 32], BF, tag="xh")
        nc.scalar.dma_start(xh_sb, ap["xh"].rearrange("(kt p) t -> p kt t", p=128))

        for s in (3, 2, 0):
            halves = load_xt(s)
            if s == 0:
                xt0 = halves
            qk_stream(s, halves)

        # lk (stream 3) token-major via PE transpose
        lk_tok = p_lk.tile([128, TT, 128], BF, tag="lk")
        for tt in range(TT):
            pt = ps2.tile([128, 128], BF, tag="ps2")
            nc.tensor.transpose(pt, qkT[:, 3, tt * 128:(tt + 1) * 128], ident)
            nc.vector.tensor_copy(lk_tok[:, tt, :], pt)

        # ---- sim/attn per group ----
        attn0, attn1 = [], []
        for g in range(NG):
            i0 = g * G
            a0 = p_a0.tile([128, 256], BF, tag="a0")
            ps = ps2.tile([128, 256], F32, tag="ps2")
            nc.tensor.matmul(ps, qkT[:, 2, i0:i0 + 128], qkT[:, 0, i0:i0 + 256],
                             start=True, stop=True)
            nc.scalar.activation(a0, ps, AF.Relu, bias=0.0, scale=1.0 / G)
            nc.vector.tensor_mul(a0[:, 0:128], a0[:, 0:128], triu)
            nc.vector.tensor_mul(a0, a0, a0)
            nc.vector.tensor_add(a0[:, 0:128], a0[:, 0:128], bdiag)
            nc.vector.tensor_add(a0[:, 128:256], a0[:, 128:256], bcorn)
            attn0.append(a0)

            a1 = p_a1.tile([128, 128], BF, tag="a1")
            ps = ps2.tile([128, 256], F32, tag="ps2")
            nc.tensor.matmul(ps[:, 0:128], qkT[:, 2, i0 + 128:i0 + 256],
                             qkT[:, 0, i0 + 128:i0 + 256], start=True, stop=True)
            nc.scalar.activation(a1, ps[:, 0:128], AF.Relu, bias=0.0, scale=1.0 / G)
            nc.vector.tensor_mul(a1, a1, triu)
            nc.vector.tensor_mul(a1, a1, a1)
            nc.vector.tensor_add(a1, a1, bdiag)
            attn1.append(a1)

        # ---- phases B-E per e-half ----
        S_full = p_sw.tile([128, NG, HID], BF, tag="sw")
        S_offb = p_sob.tile([128, HID], BF, tag="sob")

        def wsum_half(eh, t_half):
            """AllGather totals for this half, weighted-sum, add into S_full.

            Runs on GpSimd so the DVE stream never parks on the collective.
            """
            e0 = eh * EH
            cc_in, cc_out = ap[f"cc_in{eh}"], ap[f"cc_out{eh}"]
            nc.sync.dma_start(cc_in, t_half)
            nc.gpsimd.collective_compute(
                "AllGather", ALU.bypass, replica_groups=[list(range(NC))],
                ins=[cc_in], outs=[cc_out])
            for r in range(NC):
                tr = p_tr.tile([128, EH], BF, tag="tr")
                nc.sync.dma_start(tr, cc_out[r * 128:(r + 1) * 128, :])
                if r == 0:
                    nc.vector.tensor_scalar_mul(S_offb[:, e0:e0 + EH], tr,
                                                wsumw[:, 0:1])
                else:
                    nc.vector.scalar_tensor_tensor(
                        S_offb[:, e0:e0 + EH], tr, wsumw[:, r:r + 1],
                        S_offb[:, e0:e0 + EH], op0=ALU.mult, op1=ALU.add)
            nc.vector.tensor_copy(S_full[:, 0, e0:e0 + EH], S_offb[:, e0:e0 + EH])
            for g in range(1, NG):
                nc.vector.tensor_add(S_full[:, g, e0:e0 + EH],
                                     S_full[:, g, e0:e0 + EH],
                                     S_offb[:, e0:e0 + EH])

        outT = p_outT.tile([128, ET, T], BF, tag="outT")

        def lin_half(eh):
            for g in range(NG):
                for et in range(eh * 8, eh * 8 + 8):
                    po = ps2.tile([128, 256], F32, tag="ps2")
                    nc.tensor.matmul(po, S_full[:, g, et * 128:(et + 1) * 128],
                                     qkT[:, 1, g * G:(g + 1) * G],
                                     start=True, stop=True)
                    nc.vector.tensor_add(outT[:, et, g * G:(g + 1) * G],
                                         outT[:, et, g * G:(g + 1) * G], po)

        for eh in range(2):
            e0 = eh * EH
            v_h = p_big.tile([128, TT, EH], BF, tag="big")
            tails = p_tails.tile([32, NG, EH], BF, tag="tails")
            t_half = p_so.tile([128, EH], BF, tag="so")

            # v (token-major), with the kv chain interleaved after each odd
            # token tile so the AllGather can fire right after the last tile
            pk0 = pskv.tile([128, 512], F32, tag="pskv")
            pk1 = pskv.tile([128, 512], F32, tag="pskv")
            pk = [pk0, pk1]
            for tt in range(TT):
                for ec in range(2):
                    c0 = ec * 512
                    ps = ps1.tile([128, 512], F32, tag="ps1")
                    for kt in range(KD):
                        nc.tensor.matmul(
                            ps, xt_k(xt0, kt)[:, tt * 128:(tt + 1) * 128],
                            wv_sb[:, kt, e0 + c0:e0 + c0 + 512],
                            start=(kt == 0), stop=(kt == KD - 1 and not WITH_VBIAS),
                            skip_group_check=True)
                    if WITH_VBIAS:
                        nc.tensor.matmul(ps, ones_t[0:1, 0:128],
                                         wvb[0:1, e0 + c0:e0 + c0 + 512],
                                         start=False, stop=True,
                                         skip_group_check=True)
                    nc.scalar.activation(v_h[:, tt, c0:c0 + 512], ps, AF.Silu,
                                         bias=0.0, scale=1.0)
                if tt % 2 == 1:
                    g = tt // 2
                    for ec in range(2):
                        c0 = ec * 512
                        for jt in range(2):
                            nc.tensor.matmul(pk[ec], lk_tok[:, 2 * g + jt, :],
                                             v_h[:, 2 * g + jt, c0:c0 + 512],
                                             start=(g == 0 and jt == 0),
                                             stop=(g == NG - 1 and jt == 1),
                                             skip_group_check=True)
                        dst = (S_full[:, g + 1, e0 + c0:e0 + c0 + 512]
                               if g < NG - 1 else t_half[:, c0:c0 + 512])
                        nc.scalar.activation(dst, pk[ec], AF.Copy, bias=0.0,
                                             scale=1.0 / G)

            # halo -> tails[:, 0, :] (masked for core 0)
            for ec in range(2):
                c0 = ec * 512
                ps = ps1.tile([32, 512], F32, tag="ps1")
                for kt in range(KD):
                    nc.tensor.matmul(ps, xh_sb[:, kt, :],
                                     wv_sb[:, kt, e0 + c0:e0 + c0 + 512],
                                     start=(kt == 0),
                                     stop=(kt == KD - 1 and not WITH_VBIAS),
                                     skip_group_check=True)
                if WITH_VBIAS:
                    nc.tensor.matmul(ps, ones_t[0:1, 0:32],
                                     wvb[0:1, e0 + c0:e0 + c0 + 512],
                                     start=False, stop=True, skip_group_check=True)
                nc.scalar.activation(tails[:, 0, c0:c0 + 512], ps, AF.Silu,
                                     bias=0.0, scale=1.0)
                nc.vector.tensor_scalar_mul(tails[:, 0, c0:c0 + 512],
                                            tails[:, 0, c0:c0 + 512], hmask)

            # group tails (last 32 tokens of previous group)
            for g in range(1, NG):
                nc.sync.dma_start(tails[:, g, :], v_h[96:128, 2 * g - 1, :])

            # AllGather for this half as soon as totals exist
            wsum_half(eh, t_half)

            # quad + conv boundary -> outT
            for g in range(NG):
                for et in range(8):
                    ec0 = et * 128
                    po = ps2.tile([128, 256], F32, tag="ps2")
                    nc.tensor.matmul(po, v_h[:, 2 * g, ec0:ec0 + 128], attn0[g],
                                     start=True, stop=False, skip_group_check=True)
                    nc.tensor.matmul(po[:, 128:256],
                                     v_h[:, 2 * g + 1, ec0:ec0 + 128],
                                     attn1[g], start=False, stop=False,
                                     skip_group_check=True)
                    nc.tensor.matmul(po[:, 0:32], tails[:, g, ec0:ec0 + 128], bprev,
                                     start=False, stop=True, skip_group_check=True)
                    nc.scalar.activation(outT[:, eh * 8 + et, g * G:(g + 1) * G],
                                         po, AF.Copy, bias=0.0, scale=1.0)

            if DEBUG_DUMPS:
                nc.sync.dma_start(ap[f"dbg_v{eh}"], v_h)
                nc.sync.dma_start(ap[f"dbg_tails{eh}"], tails)

        # prefetch Wg (slot frees after the last Wv reader) and the first gate
        # stream's xt before fencing
        wg_sb = p_w.tile([128, KD, HID], BF, tag="w")
        for kt in range(KD):
            nc.scalar.dma_start(wg_sb[:, kt, :],
                                ap["wg"][kt * 128:(kt + 1) * 128, :])
        halves_s0 = load_xt(0)

        # scheduler fence: the lin matmuls wait on the AllGather + weighted
        # sum; without it the scheduler hoists them into the e-half-1 stream
        # and parks the PE on the collective for ~35us
        tc.no_sync_barrier()

        def gate_th(s, halves, th, defer_muls=False):
            t0 = th * 512
            og = p_big.tile([128, ET, 512], BF, tag="big")
            for et in range(ET):
                ps = ps1.tile([128, 512], F32, tag="ps1")
                for kt in range(KD):
                    nc.tensor.matmul(
                        ps, wg_sb[:, kt, et * 128:(et + 1) * 128],
                        xt_k(halves, kt)[:, t0:t0 + 512],
                        start=(kt == 0), stop=(kt == KD - 1))
                nc.scalar.activation(og[:, et, :], ps, AF.Silu,
                                     bias=bgate[:, et:et + 1], scale=1.0)
                if not defer_muls:
                    nc.vector.tensor_mul(og[:, et, :], og[:, et, :],
                                         outT[:, et, t0:t0 + 512])
            if DEBUG_DUMPS and s == 0:
                nc.sync.dma_start(ap[f"dbg_og{th}"], og)
            return og

        def og_muls(og, th):
            t0 = th * 512
            for et in range(ET):
                nc.vector.tensor_mul(og[:, et, :], og[:, et, :],
                                     outT[:, et, t0:t0 + 512])

        def y_th(s, og, th):
            for tl in range(4):
                tt = th * 4 + tl
                for nch in range(2):
                    n0 = nch * 512
                    ps = ps1.tile([128, 512], F32, tag="ps1")
                    for kt in range(ET):
                        nc.tensor.matmul(
                            ps, og[:, kt, tl * 128:(tl + 1) * 128],
                            wout_sb[:, kt, n0:n0 + 512],
                            start=(kt == 0),
                            stop=(kt == ET - 1 and not WITH_OBIAS),
                            skip_group_check=True)
                    if WITH_OBIAS:
                        nc.tensor.matmul(ps, ones_t[0:1, 0:128],
                                         bout[0:1, n0:n0 + 512],
                                         start=False, stop=True,
                                         skip_group_check=True)
                    xr = p_xr.tile([128, 512], F32, tag="xr")
                    nc.sync.dma_start(
                        xr, ap["xtok"][s, tt * 128:(tt + 1) * 128, n0:n0 + 512])
                    ysb = p_y.tile([128, 512], F32, tag="ysb")
                    nc.vector.scalar_tensor_tensor(
                        ysb, ps, 0.0, xr, op0=ALU.add, op1=ALU.add)
                    nc.sync.dma_start(
                        ap["y"][s, tt * 128:(tt + 1) * 128, n0:n0 + 512], ysb)

        # lin joined via DVE add (og-muls read outT, so all lin adds must be
        # emitted before any gate block)
        lin_half(0)
        lin_half(1)

        if DEBUG_DUMPS:
            nc.sync.dma_start(ap["dbg_qkT"], qkT)
            nc.sync.dma_start(ap["dbg_outT"], outT)
            nc.sync.dma_start(ap["dbg_sfull"], S_full)
            for g in range(NG):
                nc.sync.dma_start(ap["dbg_attn0"][g], attn0[g])

        # ---- gate + y per stream ----
        wout_sb = p_sw.tile([128, ET, DIM], BF, tag="sw")
        for kt in range(ET):
            nc.scalar.dma_start(wout_sb[:, kt, :],
                                ap["wout"][kt * 128:(kt + 1) * 128, :])

        for s in range(4):
            halves = halves_s0 if s == 0 else load_xt(s)
            for th in range(2):
                og = gate_th(s, halves, th)
                y_th(s, og, th)


def build_nc(with_vbias=None, with_obias=None):
    global WITH_VBIAS, WITH_OBIAS
    if with_vbias is not None:
        WITH_VBIAS = with_vbias
    if with_obias is not None:
        WITH_OBIAS = with_obias
    nc = bacc.Bacc("TRN2", target_bir_lowering=False, debug=False, num_devices=NC)
    ap = {}

    def dram(name, shape, dt, kind=None, addr_space=None):
        kw = {}
        if kind:
            kw["kind"] = kind
        if addr_space:
            kw["addr_space"] = addr_space
        ap[name] = nc.dram_tensor(name, shape, dt, **kw).ap()

    dram("xt", [4, DIM, T], BF, kind="ExternalInput")
    dram("xh", [DIM, 32], BF, kind="ExternalInput")
    dram("xtok", [4, T, DIM], F32, kind="ExternalInput")
    dram("wv", [DIM, HID], BF, kind="ExternalInput")
    dram("wg", [DIM, HID], BF, kind="ExternalInput")
    dram("wqk", [DIM, DQK], BF, kind="ExternalInput")
    dram("wout", [HID, DIM], BF, kind="ExternalInput")
    dram("wvb", [1, HID], BF, kind="ExternalInput")
    dram("bout", [1, DIM], BF, kind="ExternalInput")
    dram("bgate", [128, ET], F32, kind="ExternalInput")
    dram("bqk", [128, 1], F32, kind="ExternalInput")
    dram("triu", [128, 128], BF, kind="ExternalInput")
    dram("bdiag", [128, 128], BF, kind="ExternalInput")
    dram("bcorn", [128, 128], BF, kind="ExternalInput")
    dram("bprev", [32, 32], BF, kind="ExternalInput")
    dram("hmask", [32, 1], F32, kind="ExternalInput")
    dram("wsumw", [128, NC], F32, kind="ExternalInput")
    if DEBUG_DUMPS:
        dram("dbg_v0", [128, TT, EH], BF, kind="ExternalOutput")
        dram("dbg_v1", [128, TT, EH], BF, kind="ExternalOutput")
        dram("dbg_tails0", [32, NG, EH], BF, kind="ExternalOutput")
        dram("dbg_tails1", [32, NG, EH], BF, kind="ExternalOutput")
        dram("dbg_qkT", [128, 4, T], BF, kind="ExternalOutput")
        dram("dbg_outT", [128, ET, T], BF, kind="ExternalOutput")
        dram("dbg_sfull", [128, NG, HID], BF, kind="ExternalOutput")
        dram("dbg_attn0", [NG, 128, 256], BF, kind="ExternalOutput")
        dram("dbg_og0", [128, ET, 512], BF, kind="ExternalOutput")
        dram("dbg_og1", [128, ET, 512], BF, kind="ExternalOutput")
    dram("cc_warm_in", [128, 16], BF)
    dram("cc_warm_out", [NC * 128, 16], BF, addr_space="Shared")
    dram("cc_in0", [128, EH], BF)
    dram("cc_out0", [NC * 128, EH], BF, addr_space="Shared")
    dram("cc_in1", [128, EH], BF)
    dram("cc_out1", [NC * 128, EH], BF, addr_space="Shared")
    dram("y", [4, T, DIM], F32, kind="ExternalOutput")

    with tile.TileContext(nc) as tc:
        _emit(tc, ap)
    nc.compile()
    return nc


def host_prep(inputs):
    """Pure layout transforms: shard, transpose, cast, build conv-band consts."""
    x = np.ascontiguousarray(np.asarray(inputs["x"], np.float32)[0])  # [4, N, DIM]
    W_h = np.asarray(inputs["W_h"], np.float32)
    b_h = np.asarray(inputs["b_h"], np.float32)
    W_qk = np.asarray(inputs["W_qk"], np.float32)
    b_qk = np.asarray(inputs["b_qk"], np.float32)
    W_out = np.asarray(inputs["W_out"], np.float32)
    b_out = np.asarray(inputs["b_out"], np.float32)
    cw = np.asarray(inputs["conv_w"], np.float32)

    jj = np.arange(128)[:, None]
    ii = np.arange(128)[None, :]
    d = ii - jj
    triu = (ii >= jj).astype(bf16)
    bdiag = np.where((d >= 0) & (d <= 31), cw[np.clip(31 - d, 0, 62)], 0.0).astype(bf16)
    dc = (ii + 128) - jj
    bcorn = np.where((dc >= 0) & (dc <= 31),
                     cw[np.clip(31 - dc, 0, 62)], 0.0).astype(bf16)
    jt = np.arange(32)[:, None]
    ip = np.arange(32)[None, :]
    dp = ip + 32 - jt
    bprev = np.where((dp >= 1) & (dp <= 31),
                     cw[np.clip(31 - dp, 0, 62)], 0.0).astype(bf16)

    common = {
        "wv": np.ascontiguousarray(W_h[:, :HID]).astype(bf16),
        "wg": np.ascontiguousarray(W_h[:, HID:]).astype(bf16),
        "wqk": W_qk.astype(bf16),
        "wout": W_out.astype(bf16),
        "wvb": b_h[None, :HID].astype(bf16),
        "bout": b_out[None, :].astype(bf16),
        "bgate": np.ascontiguousarray(b_h[HID:].reshape(ET, 128).T).astype(np.float32),
        "bqk": b_qk[:, None].astype(np.float32),
        "triu": triu, "bdiag": bdiag, "bcorn": bcorn, "bprev": bprev,
    }

    in_maps = []
    for c in range(NC):
        sl = slice(c * T, (c + 1) * T)
        x_c = x[:, sl, :]
        xt = np.zeros((4, DIM, T), bf16)
        for s in range(4):
            xt[s] = x_c[s].T.astype(bf16)
        if c > 0:
            xh = np.ascontiguousarray(x[0, c * T - 32:c * T, :].T).astype(bf16)
        else:
            xh = np.zeros((DIM, 32), bf16)
        m = dict(common)
        m["xt"] = xt
        m["xh"] = xh
        m["xtok"] = np.ascontiguousarray(x_c)
        m["hmask"] = np.full((32, 1), 1.0 if c > 0 else 0.0, np.float32)
        w = np.zeros((128, NC), np.float32)
        w[:, :c] = 1.0
        m["wsumw"] = w
        in_maps.append(m)
    return in_maps


_NC_PROG = None
_NC_FLAGS = None


def kernel(**inputs):
    global _NC_PROG, _NC_FLAGS
    b_h = np.asarray(inputs["b_h"], np.float32)
    b_out = np.asarray(inputs["b_out"], np.float32)
    flags = (bool(np.any(b_h[:HID])), bool(np.any(b_out)))
    if _NC_PROG is None or _NC_FLAGS != flags:
        _NC_PROG = build_nc(with_vbias=flags[0], with_obias=flags[1])
        _NC_FLAGS = flags
    in_maps = host_prep(inputs)
    res = run_bass_kernel_spmd(_NC_PROG, in_maps, list(range(NC)))
    y = np.stack([res.results[c]["y"] for c in range(NC)], axis=1)  # [4, NC, T, DIM]
    return np.ascontiguousarray(y.reshape(4, NSEQ, DIM)[None]).astype(np.float32)



# revision 8
# speedup vs baseline: 1.0067x; 1.0067x over previous
"""Trainium2 Bass kernel for nn_FLASH_40458591928592 (sparse_attention).

Sequence-sharded over 8 NeuronCores: 1024 tokens (= 4 groups of 256) per core.
Mixed precision, validated against a numpy e4m3 simulation (rel 1.66e-2 < 2e-2):
  qk GEMM : fully fp8 DoubleRow (x*0.25 stationary-free scales, W*4)
  v GEMM  : fully fp8 DoubleRow; v_h and tails stored fp8 (storage only)
  gate    : k-tiles 0-1 of 8 fp8 DR, rest bf16 (same PSUM, products at scale 1)
  y       : HID k-tiles 0-3 of 16 fp8 DR (og et 0-3 written fp8*0.25 by DVE)

Phase order keeps the PE dense and the HAM clock warm:
  dummy warmup matmuls through the DMA lead-in -> v -> qk(3,2,0,1) -> sim/attn
  -> lk transposes -> kv chains (AllGather per e-half fires ~60us) -> quad+conv
  (overlapping the collectives) -> lin -> gate+y per stream.
One LDWEIGHTS feeds 2-4 matmuls everywhere (ec/ch/th/nch pairing).
SBUF is tag-chained across serial phases (vh->ogb, wv8->woutb, qkT->gt,
S_full->og8, t_half->wg8, lk->wout8, S_offb->xr, tails->ysb).
"""

from contextlib import ExitStack

import numpy as np
import ml_dtypes

import concourse.tile as tile
from concourse import bacc, mybir
from concourse.bass_utils import run_bass_kernel_spmd
from concourse.masks import make_identity

BF = mybir.dt.bfloat16
F8 = mybir.dt.float8e4
F32 = mybir.dt.float32
bf16 = ml_dtypes.bfloat16
fp8 = ml_dtypes.float8_e4m3
DRM = mybir.MatmulPerfMode.DoubleRow

G = 256
DIM = 1024
HID = 2048
DQK = 128
NSEQ = 8192
NC = 8
T = NSEQ // NC        # 1024 tokens per core
NG = T // G           # 4 groups per core
KD = DIM // 128       # 8 k-tiles over dim
ET = HID // 128       # 16 e-tiles over hid
TT = T // 128         # 8 token tiles
EH = HID // 2         # 1024 cols per e-half

KG8 = 2               # gate fp8 k-tiles (of KD); one DR pair
KY8 = 4               # y fp8 k-tiles (of ET); must be even
KB = KD - KG8         # bf16 k-tiles for gate
XS = 0.25             # fp8 x-side scale
WS = 4.0              # fp8 w-side scale (XS*WS == 1 -> shared-PSUM)
NWARM = 70            # HAM warmup dummy matmuls

AF = mybir.ActivationFunctionType
ALU = mybir.AluOpType

DEBUG_DUMPS = False
WITH_VBIAS = True
WITH_OBIAS = True


def _emit(tc, ap):
    nc = tc.nc
    with ExitStack() as ctx:
        consts = ctx.enter_context(tc.tile_pool(name="consts", bufs=1))
        p_x8 = ctx.enter_context(tc.tile_pool(name="x8", bufs=2))
        p_xtb = ctx.enter_context(tc.tile_pool(name="xtb", bufs=3))
        p_x08 = ctx.enter_context(tc.tile_pool(name="x08", bufs=1))
        p_big = ctx.enter_context(tc.tile_pool(name="big", bufs=1))   # vh8 -> ogb
        p_qog = ctx.enter_context(tc.tile_pool(name="qog", bufs=1))   # qkT -> gt
        p_lk = ctx.enter_context(tc.tile_pool(name="lk", bufs=1))     # lk -> wout8
        p_w1 = ctx.enter_context(tc.tile_pool(name="w1", bufs=1))     # wv8 -> woutb
        p_w2 = ctx.enter_context(tc.tile_pool(name="w2", bufs=1))     # wgb
        p_tails = ctx.enter_context(tc.tile_pool(name="tails", bufs=1))  # -> ysb
        p_so = ctx.enter_context(tc.tile_pool(name="so", bufs=2))     # t_half -> wg8
        p_sob = ctx.enter_context(tc.tile_pool(name="sob", bufs=1))   # S_offb -> xr
        p_sf = ctx.enter_context(tc.tile_pool(name="sf", bufs=1))     # S_full -> og8
        p_tr = ctx.enter_context(tc.tile_pool(name="tr", bufs=1))
        p_a0 = ctx.enter_context(tc.tile_pool(name="a0", bufs=4))
        p_a1 = ctx.enter_context(tc.tile_pool(name="a1", bufs=4))
        p_outT = ctx.enter_context(tc.tile_pool(name="outT", bufs=1))
        ps1 = ctx.enter_context(tc.tile_pool(name="ps1", bufs=4, space="PSUM"))
        ps2 = ctx.enter_context(tc.tile_pool(name="ps2", bufs=2, space="PSUM"))
        pskv = ctx.enter_context(tc.tile_pool(name="pskv", bufs=2, space="PSUM"))

        # ---- HAM warmup: keep PE busy through the DMA lead-in ----
        ident = consts.tile([128, 128], BF, tag="ident")
        make_identity(nc, ident)
        for _ in range(NWARM):
            pw = ps2.tile([128, 128], F32, tag="ps2", name="pw")
            nc.tensor.matmul(pw, ident, ident, start=True, stop=True)

        # ---- first DMAs: v-GEMM inputs, then qk weights ----
        x08 = p_x08.tile([128, KD, T], F8, tag="x08")
        nc.sync.dma_start(x08, ap["xt08"].rearrange("(kt p) t -> p kt t", p=128))
        wv8 = p_w1.tile([128, KD, HID], F8, tag="w1")
        nc.scalar.dma_start(wv8, ap["wv8"].rearrange("(kt p) e -> p kt e", p=128))
        bqk = consts.tile([128, 1], F32, tag="bqk")
        nc.scalar.dma_start(bqk, ap["bqk"])
        wqk8 = consts.tile([128, KD, DQK], F8, tag="wqk8")
        nc.scalar.dma_start(wqk8, ap["wqk8"].rearrange("(kt p) q -> p kt q", p=128))
        xh8 = consts.tile([128, KD, 32], F8, tag="xh8")
        nc.scalar.dma_start(xh8, ap["xh8"].rearrange("(kt p) t -> p kt t", p=128))

        x8f = {0: x08}

        def load_x8f(s):
            t8 = p_x8.tile([128, KD, T], F8, tag="x8", name=f"x8_{s}")
            nc.sync.dma_start(t8, ap["xt8"][s - 1].rearrange("(kt p) t -> p kt t",
                                                             p=128))
            x8f[s] = t8

        load_x8f(3)
        load_x8f(2)

        # warm-up collective: absorbs first-collective setup latency
        cwarm = consts.tile([128, 16], BF, tag="cwarm")
        nc.vector.memset(cwarm, 0.0)
        nc.sync.dma_start(ap["cc_warm_in"], cwarm)
        nc.gpsimd.collective_compute(
            "AllGather", ALU.bypass, replica_groups=[list(range(NC))],
            ins=[ap["cc_warm_in"]], outs=[ap["cc_warm_out"]])

        # remaining consts (DMA behind weights on scalar queue)
        triu = consts.tile([128, 128], BF, tag="triu")
        nc.scalar.dma_start(triu, ap["triu"])
        bdiag = consts.tile([128, 128], BF, tag="bdiag")
        nc.scalar.dma_start(bdiag, ap["bdiag"])
        bcorn = consts.tile([128, 128], BF, tag="bcorn")
        nc.scalar.dma_start(bcorn, ap["bcorn"])
        bprev = consts.tile([32, 32], BF, tag="bprev")
        nc.scalar.dma_start(bprev, ap["bprev"])
        hmask = consts.tile([32, 1], F32, tag="hmask")
        nc.scalar.dma_start(hmask, ap["hmask"])
        wsumw = consts.tile([128, NC], F32, tag="wsumw")
        nc.scalar.dma_start(wsumw, ap["wsumw"])
        bgate = consts.tile([128, ET], F32, tag="bgate")
        nc.scalar.dma_start(bgate, ap["bgate"])
        if WITH_VBIAS or WITH_OBIAS:
            ones_t = consts.tile([1, 1024], BF, tag="ones")
            nc.vector.memset(ones_t, 1.0)
        if WITH_VBIAS:
            wvb = consts.tile([1, HID], BF, tag="wvb")
            nc.scalar.dma_start(wvb, ap["wvb"])
        if WITH_OBIAS:
            bout = consts.tile([1, DIM], BF, tag="bout")
            nc.scalar.dma_start(bout, ap["bout"])

        # ---- v GEMM: fp8 DR, one xt-pair LDWEIGHTS feeds 4 e-chunks ----
        v_h = p_big.tile([128, TT, HID], F8, tag="big", name="v_h")
        for tt in range(TT):
            pv = [ps1.tile([128, 512], F32, tag="ps1", name="pv") for _ in range(4)]
            for kp in range(KD // 2):
                for ec in range(4):
                    nc.tensor.matmul(
                        pv[ec], x08[:, 2 * kp:2 * kp + 2, tt * 128:(tt + 1) * 128],
                        wv8[:, 2 * kp:2 * kp + 2, ec * 512:(ec + 1) * 512],
                        start=(kp == 0),
                        stop=(kp == KD // 2 - 1 and not WITH_VBIAS),
                        perf_mode=DRM, skip_group_check=True)
            if WITH_VBIAS:
                for ec in range(4):
                    nc.tensor.matmul(pv[ec], ones_t[0:1, 0:128],
                                     wvb[0:1, ec * 512:(ec + 1) * 512],
                                     start=False, stop=True, skip_group_check=True)
            for ec in range(4):
                nc.scalar.activation(v_h[:, tt, ec * 512:(ec + 1) * 512], pv[ec],
                                     AF.Silu, bias=0.0, scale=1.0)

        # halo: last 32 tokens of the previous core (masked on core 0)
        tails = p_tails.tile([32, NG, HID], F8, tag="tails")
        ph = [ps1.tile([32, 512], F32, tag="ps1", name="ph") for _ in range(4)]
        for kp in range(KD // 2):
            for ec in range(4):
                nc.tensor.matmul(
                    ph[ec], xh8[:, 2 * kp:2 * kp + 2, :],
                    wv8[:, 2 * kp:2 * kp + 2, ec * 512:(ec + 1) * 512],
                    start=(kp == 0), stop=(kp == KD // 2 - 1 and not WITH_VBIAS),
                    perf_mode=DRM, skip_group_check=True)
        if WITH_VBIAS:
            for ec in range(4):
                nc.tensor.matmul(ph[ec], ones_t[0:1, 0:32],
                                 wvb[0:1, ec * 512:(ec + 1) * 512],
                                 start=False, stop=True, skip_group_check=True)
        for ec in range(4):
            nc.scalar.activation(tails[:, 0, ec * 512:(ec + 1) * 512], ph[ec],
                                 AF.Silu, bias=0.0, scale=1.0)
            nc.vector.tensor_scalar_mul(tails[:, 0, ec * 512:(ec + 1) * 512],
                                        tails[:, 0, ec * 512:(ec + 1) * 512], hmask)
        for g in range(1, NG):
            nc.scalar.dma_start(tails[:, g, :], v_h[96:128, 2 * g - 1, :])

        # ---- qk streams: fully fp8 DR, ch-paired ----
        qkT = p_qog.tile([128, 4, T], BF, tag="qog", name="qkT")
        for si, s in enumerate((3, 2, 0, 1)):
            pc = [ps1.tile([128, 512], F32, tag="ps1", name="pc") for _ in range(2)]
            for kp in range(KD // 2):
                for ch in range(2):
                    nc.tensor.matmul(pc[ch], wqk8[:, 2 * kp:2 * kp + 2, :],
                                     x8f[s][:, 2 * kp:2 * kp + 2,
                                            ch * 512:(ch + 1) * 512],
                                     start=(kp == 0), stop=(kp == KD // 2 - 1),
                                     perf_mode=DRM, skip_group_check=True)
            for ch in range(2):
                nc.scalar.activation(qkT[:, s, ch * 512:(ch + 1) * 512], pc[ch],
                                     AF.Silu, bias=bqk, scale=1.0)
            if si == 0:
                load_x8f(1)   # slot rotation WARs on qk3's reads

        # ---- sim/attn per group (conv band folded into bdiag/bcorn) ----
        attn0, attn1 = [], []
        for g in range(NG):
            i0 = g * G
            a0 = p_a0.tile([128, 256], BF, tag="a0")
            ps = ps2.tile([128, 256], F32, tag="ps2")
            nc.tensor.matmul(ps, qkT[:, 2, i0:i0 + 128], qkT[:, 0, i0:i0 + 256],
                             start=True, stop=True)
            nc.scalar.activation(a0, ps, AF.Relu, bias=0.0, scale=1.0 / G)
            nc.vector.tensor_mul(a0[:, 0:128], a0[:, 0:128], triu)
            nc.vector.tensor_mul(a0, a0, a0)
            nc.vector.tensor_add(a0[:, 0:128], a0[:, 0:128], bdiag)
            nc.vector.tensor_add(a0[:, 128:256], a0[:, 128:256], bcorn)
            attn0.append(a0)

            a1 = p_a1.tile([128, 128], BF, tag="a1")
            ps = ps2.tile([128, 256], F32, tag="ps2")
            nc.tensor.matmul(ps[:, 0:128], qkT[:, 2, i0 + 128:i0 + 256],
                             qkT[:, 0, i0 + 128:i0 + 256], start=True, stop=True)
            nc.scalar.activation(a1, ps[:, 0:128], AF.Relu, bias=0.0, scale=1.0 / G)
            nc.vector.tensor_mul(a1, a1, triu)
            nc.vector.tensor_mul(a1, a1, a1)
            nc.vector.tensor_add(a1, a1, bdiag)
            attn1.append(a1)

        # lk (stream 3) token-major via PE transpose
        lk_tok = p_lk.tile([128, TT, 128], BF, tag="lk", name="lk_tok")
        for tt in range(TT):
            pt = ps2.tile([128, 128], BF, tag="ps2", name="pt")
            nc.tensor.transpose(pt, qkT[:, 3, tt * 128:(tt + 1) * 128], ident)
            nc.vector.tensor_copy(lk_tok[:, tt, :], pt)

        # ---- kv chains + AllGather per e-half ----
        S_full = p_sf.tile([128, NG, HID], BF, tag="sf", name="S_full")
        S_offb = p_sob.tile([128, HID], BF, tag="sob", name="S_offb")

        def wsum_half(eh, t_half):
            e0 = eh * EH
            cc_in, cc_out = ap[f"cc_in{eh}"], ap[f"cc_out{eh}"]
            nc.scalar.dma_start(cc_in, t_half)
            nc.gpsimd.collective_compute(
                "AllGather", ALU.bypass, replica_groups=[list(range(NC))],
                ins=[cc_in], outs=[cc_out])
            for r in range(NC):
                tr = p_tr.tile([128, EH], BF, tag="tr")
                nc.sync.dma_start(tr, cc_out[r * 128:(r + 1) * 128, :])
                if r == 0:
                    nc.vector.tensor_scalar_mul(S_offb[:, e0:e0 + EH], tr,
                                                wsumw[:, 0:1])
                else:
                    nc.vector.scalar_tensor_tensor(
                        S_offb[:, e0:e0 + EH], tr, wsumw[:, r:r + 1],
                        S_offb[:, e0:e0 + EH], op0=ALU.mult, op1=ALU.add)
            nc.vector.tensor_copy(S_full[:, 0, e0:e0 + EH], S_offb[:, e0:e0 + EH])
            for g in range(1, NG):
                nc.vector.tensor_add(S_full[:, g, e0:e0 + EH],
                                     S_full[:, g, e0:e0 + EH],
                                     S_offb[:, e0:e0 + EH])

        for eh in range(2):
            e0 = eh * EH
            t_half = p_so.tile([128, EH], BF, tag="so", name="t_half")
            pk = [pskv.tile([128, 512], F32, tag="pskv", name="pk")
                  for _ in range(2)]
            for g in range(NG):
                for jt in range(2):
                    for ec in range(2):
                        nc.tensor.matmul(
                            pk[ec], lk_tok[:, 2 * g + jt, :],
                            v_h[:, 2 * g + jt, e0 + ec * 512:e0 + (ec + 1) * 512],
                            start=(g == 0 and jt == 0),
                            stop=(g == NG - 1 and jt == 1),
                            skip_group_check=True)
                for ec in range(2):
                    dst = (S_full[:, g + 1, e0 + ec * 512:e0 + (ec + 1) * 512]
                           if g < NG - 1 else t_half[:, ec * 512:(ec + 1) * 512])
                    nc.scalar.activation(dst, pk[ec], AF.Copy, bias=0.0,
                                         scale=1.0 / G)
            wsum_half(eh, t_half)

        # ---- quad + conv boundary -> outT ----
        outT = p_outT.tile([128, ET, T], BF, tag="outT")
        for eh in range(2):
            e0 = eh * EH
            for g in range(NG):
                for et in range(8):
                    ec0 = e0 + et * 128
                    po = ps2.tile([128, 256], F32, tag="ps2", name="po")
                    nc.tensor.matmul(po, v_h[:, 2 * g, ec0:ec0 + 128], attn0[g],
                                     start=True, stop=False, skip_group_check=True)
                    nc.tensor.matmul(po[:, 128:256],
                                     v_h[:, 2 * g + 1, ec0:ec0 + 128],
                                     attn1[g], start=False, stop=False,
                                     skip_group_check=True)
                    nc.tensor.matmul(po[:, 0:32], tails[:, g, ec0:ec0 + 128], bprev,
                                     start=False, stop=True, skip_group_check=True)
                    nc.scalar.activation(outT[:, eh * 8 + et, g * G:(g + 1) * G],
                                         po, AF.Copy, bias=0.0, scale=1.0)

        # gate weights (DMA while PE chews on quad)
        wgb = p_w2.tile([128, KB, HID], BF, tag="w2", name="wgb")
        for kt in range(KB):
            nc.scalar.dma_start(wgb[:, kt, :],
                                ap["wgb"][kt * 128:(kt + 1) * 128, :])
        wg8 = p_so.tile([128, KG8, HID], F8, tag="so", name="wg8")
        nc.scalar.dma_start(wg8, ap["wg8"].rearrange("(kt p) e -> p kt e", p=128))

        # ---- lin joined via DVE add into outT ----
        for eh in range(2):
            for g in range(NG):
                for et in range(eh * 8, eh * 8 + 8):
                    po = ps2.tile([128, 256], F32, tag="ps2", name="po")
                    nc.tensor.matmul(po, S_full[:, g, et * 128:(et + 1) * 128],
                                     qkT[:, 1, g * G:(g + 1) * G],
                                     start=True, stop=True)
                    nc.vector.tensor_add(outT[:, et, g * G:(g + 1) * G],
                                         outT[:, et, g * G:(g + 1) * G], po)

        # out-projection weights (DMA during lin/first gate)
        woutb = p_w1.tile([128, ET - KY8, DIM], BF, tag="w1", name="woutb")
        for kt in range(ET - KY8):
            nc.scalar.dma_start(woutb[:, kt, :],
                                ap["woutb"][kt * 128:(kt + 1) * 128, :])
        wout8 = p_lk.tile([128, KY8, DIM], F8, tag="lk", name="wout8")
        nc.scalar.dma_start(wout8, ap["wout8"].rearrange("(kt p) n -> p kt n",
                                                         p=128))

        if DEBUG_DUMPS:
            nc.sync.dma_start(ap["dbg_qkT"], qkT)
            nc.sync.dma_start(ap["dbg_vh"], v_h)
            nc.sync.dma_start(ap["dbg_outT"], outT)
            nc.sync.dma_start(ap["dbg_sfull"], S_full)

        # bf16 gate inputs (k-tiles 2..7), loaded during lin / earlier streams
        xtb = {}

        def load_xtb(s):
            halves = []
            for q in range(2):
                h = p_xtb.tile([128, KB // 2, T], BF, tag="xtb", name=f"xtb{s}_{q}")
                nc.sync.dma_start(
                    h, ap["xtb"][s, q * 384:(q + 1) * 384, :].rearrange(
                        "(kt p) t -> p kt t", p=128))
                halves.append(h)
            xtb[s] = halves

        load_xtb(0)
        load_xtb(1)

        # ---- gate + y per stream (th-paired gate, nch-paired y) ----
        for s in range(4):
            og8 = p_sf.tile([128, KY8, T], F8, tag="sf", name="og8")
            ogb = p_big.tile([128, ET - KY8, T], BF, tag="big", name="ogb")
            for et in range(ET):
                pg = [ps1.tile([128, 512], F32, tag="ps1", name="pg")
                      for _ in range(2)]
                for th in range(2):
                    nc.tensor.matmul(
                        pg[th], wg8[:, 0:KG8, et * 128:(et + 1) * 128],
                        x8f[s][:, 0:KG8, th * 512:(th + 1) * 512],
                        start=True, stop=False,
                        perf_mode=DRM, skip_group_check=True)
                for kt in range(KB):
                    xs_t = xtb[s][kt // 3][:, kt % 3, :]
                    for th in range(2):
                        nc.tensor.matmul(
                            pg[th], wgb[:, kt, et * 128:(et + 1) * 128],
                            xs_t[:, th * 512:(th + 1) * 512],
                            start=False, stop=(kt == KB - 1),
                            skip_group_check=True)
                for th in range(2):
                    t0 = th * 512
                    if et < KY8:
                        gt = p_qog.tile([128, 512], BF, tag="qog", name="gt")
                        nc.scalar.activation(gt, pg[th], AF.Silu,
                                             bias=bgate[:, et:et + 1], scale=1.0)
                        nc.vector.scalar_tensor_tensor(
                            og8[:, et, t0:t0 + 512], gt, XS,
                            outT[:, et, t0:t0 + 512], op0=ALU.mult, op1=ALU.mult)
                    else:
                        eb = et - KY8
                        nc.scalar.activation(ogb[:, eb, t0:t0 + 512], pg[th],
                                             AF.Silu, bias=bgate[:, et:et + 1],
                                             scale=1.0)
                        nc.vector.tensor_mul(ogb[:, eb, t0:t0 + 512],
                                             ogb[:, eb, t0:t0 + 512],
                                             outT[:, et, t0:t0 + 512])
            if s < 2:
                load_xtb(s + 2)
            if s == 0:
                load_x8f(1)
            elif s == 1:
                load_x8f(2)
            elif s == 2:
                load_x8f(3)
            for tl in range(TT):
                tsl = slice(tl * 128, (tl + 1) * 128)
                pn = [ps1.tile([128, 512], F32, tag="ps1", name="pn")
                      for _ in range(2)]
                for kp in range(KY8 // 2):
                    for nch in range(2):
                        nc.tensor.matmul(
                            pn[nch], og8[:, 2 * kp:2 * kp + 2, tsl],
                            wout8[:, 2 * kp:2 * kp + 2,
                                  nch * 512:(nch + 1) * 512],
                            start=(kp == 0), stop=False,
                            perf_mode=DRM, skip_group_check=True)
                for kt in range(ET - KY8):
                    for nch in range(2):
                        nc.tensor.matmul(
                            pn[nch], ogb[:, kt, tsl],
                            woutb[:, kt, nch * 512:(nch + 1) * 512],
                            start=False,
                            stop=(kt == ET - KY8 - 1 and not WITH_OBIAS),
                            skip_group_check=True)
                if WITH_OBIAS:
                    for nch in range(2):
                        nc.tensor.matmul(pn[nch], ones_t[0:1, 0:128],
                                         bout[0:1, nch * 512:(nch + 1) * 512],
                                         start=False, stop=True,
                                         skip_group_check=True)
                for nch in range(2):
                    n0 = nch * 512
                    xr = p_sob.tile([128, 512], F32, tag="sob", name="xr")
                    nc.sync.dma_start(xr, ap["xtok"][s, tsl, n0:n0 + 512])
                    ysb = p_tails.tile([128, 512], F32, tag="tails", name="ysb")
                    nc.vector.scalar_tensor_tensor(
                        ysb, pn[nch], 0.0, xr, op0=ALU.add, op1=ALU.add)
                    nc.sync.dma_start(ap["y"][s, tsl, n0:n0 + 512], ysb)


def build_nc(with_vbias=None, with_obias=None):
    global WITH_VBIAS, WITH_OBIAS
    if with_vbias is not None:
        WITH_VBIAS = with_vbias
    if with_obias is not None:
        WITH_OBIAS = with_obias
    nc = bacc.Bacc("TRN2", target_bir_lowering=False, debug=False, num_devices=NC)
    ap = {}

    def dram(name, shape, dt, kind=None, addr_space=None):
        kw = {}
        if kind:
            kw["kind"] = kind
        if addr_space:
            kw["addr_space"] = addr_space
        ap[name] = nc.dram_tensor(name, shape, dt, **kw).ap()

    dram("xtb", [4, KB * 128, T], BF, kind="ExternalInput")
    dram("xt8", [3, DIM, T], F8, kind="ExternalInput")
    dram("xt08", [DIM, T], F8, kind="ExternalInput")
    dram("xh8", [DIM, 32], F8, kind="ExternalInput")
    dram("xtok", [4, T, DIM], F32, kind="ExternalInput")
    dram("wv8", [DIM, HID], F8, kind="ExternalInput")
    dram("wgb", [KB * 128, HID], BF, kind="ExternalInput")
    dram("wg8", [KG8 * 128, HID], F8, kind="ExternalInput")
    dram("wqk8", [DIM, DQK], F8, kind="ExternalInput")
    dram("woutb", [(ET - KY8) * 128, DIM], BF, kind="ExternalInput")
    dram("wout8", [KY8 * 128, DIM], F8, kind="ExternalInput")
    dram("wvb", [1, HID], BF, kind="ExternalInput")
    dram("bout", [1, DIM], BF, kind="ExternalInput")
    dram("bgate", [128, ET], F32, kind="ExternalInput")
    dram("bqk", [128, 1], F32, kind="ExternalInput")
    dram("triu", [128, 128], BF, kind="ExternalInput")
    dram("bdiag", [128, 128], BF, kind="ExternalInput")
    dram("bcorn", [128, 128], BF, kind="ExternalInput")
    dram("bprev", [32, 32], BF, kind="ExternalInput")
    dram("hmask", [32, 1], F32, kind="ExternalInput")
    dram("wsumw", [128, NC], F32, kind="ExternalInput")
    if DEBUG_DUMPS:
        dram("dbg_qkT", [128, 4, T], BF, kind="ExternalOutput")
        dram("dbg_vh", [128, TT, HID], F8, kind="ExternalOutput")
        dram("dbg_outT", [128, ET, T], BF, kind="ExternalOutput")
        dram("dbg_sfull", [128, NG, HID], BF, kind="ExternalOutput")
    dram("cc_warm_in", [128, 16], BF)
    dram("cc_warm_out", [NC * 128, 16], BF, addr_space="Shared")
    dram("cc_in0", [128, EH], BF)
    dram("cc_out0", [NC * 128, EH], BF, addr_space="Shared")
    dram("cc_in1", [128, EH], BF)
    dram("cc_out1", [NC * 128, EH], BF, addr_space="Shared")
    dram("y", [4, T, DIM], F32, kind="ExternalOutput")

    with tile.TileContext(nc) as tc:
        _emit(tc, ap)
    nc.compile()
    return nc


def _f8(a):
    return np.clip(a, -240.0, 240.0).astype(fp8)


def host_prep(inputs):
    """Pure layout transforms: shard, transpose, cast, build conv-band consts."""
    x = np.ascontiguousarray(np.asarray(inputs["x"], np.float32)[0])  # [4, N, DIM]
    W_h = np.asarray(inputs["W_h"], np.float32)
    b_h = np.asarray(inputs["b_h"], np.float32)
    W_qk = np.asarray(inputs["W_qk"], np.float32)
    b_qk = np.asarray(inputs["b_qk"], np.float32)
    W_out = np.asarray(inputs["W_out"], np.float32)
    b_out = np.asarray(inputs["b_out"], np.float32)
    cw = np.asarray(inputs["conv_w"], np.float32)

    jj = np.arange(128)[:, None]
    ii = np.arange(128)[None, :]
    d = ii - jj
    triu = (ii >= jj).astype(bf16)
    bdiag = np.where((d >= 0) & (d <= 31), cw[np.clip(31 - d, 0, 62)], 0.0).astype(bf16)
    dc = (ii + 128) - jj
    bcorn = np.where((dc >= 0) & (dc <= 31),
                     cw[np.clip(31 - dc, 0, 62)], 0.0).astype(bf16)
    jt = np.arange(32)[:, None]
    ip = np.arange(32)[None, :]
    dp = ip + 32 - jt
    bprev = np.where((dp >= 1) & (dp <= 31),
                     cw[np.clip(31 - dp, 0, 62)], 0.0).astype(bf16)

    kb0 = KG8 * 128
    ky0 = KY8 * 128
    common = {
        "wv8": _f8(W_h[:, :HID] * WS),
        "wgb": np.ascontiguousarray(W_h[kb0:, HID:]).astype(bf16),
        "wg8": _f8(W_h[:kb0, HID:] * WS),
        "wqk8": _f8(W_qk * WS),
        "woutb": np.ascontiguousarray(W_out[ky0:, :]).astype(bf16),
        "wout8": _f8(W_out[:ky0, :] * WS),
        "wvb": b_h[None, :HID].astype(bf16),
        "bout": b_out[None, :].astype(bf16),
        "bgate": np.ascontiguousarray(b_h[HID:].reshape(ET, 128).T).astype(np.float32),
        "bqk": b_qk[:, None].astype(np.float32),
        "triu": triu, "bdiag": bdiag, "bcorn": bcorn, "bprev": bprev,
    }

    in_maps = []
    for c in range(NC):
        sl = slice(c * T, (c + 1) * T)
        x_c = x[:, sl, :]
        xtb = np.zeros((4, KB * 128, T), bf16)
        xt8 = np.zeros((3, DIM, T), fp8)
        for s in range(4):
            xT = x_c[s].T
            xtb[s] = xT[kb0:].astype(bf16)
            if s > 0:
                xt8[s - 1] = _f8(xT * XS)
        xt08 = _f8(x_c[0].T * XS)
        if c > 0:
            xh8 = _f8(np.ascontiguousarray(x[0, c * T - 32:c * T, :].T) * XS)
        else:
            xh8 = np.zeros((DIM, 32), fp8)
        m = dict(common)
        m["xtb"] = xtb
        m["xt8"] = xt8
        m["xt08"] = xt08
        m["xh8"] = xh8
        m["xtok"] = np.ascontiguousarray(x_c)
        m["hmask"] = np.full((32, 1), 1.0 if c > 0 else 0.0, np.float32)
        w = np.zeros((128, NC), np.float32)
        w[:, :c] = 1.0
        m["wsumw"] = w
        in_maps.append(m)
    return in_maps


_NC_PROG = None
_NC_FLAGS = None


def kernel(**inputs):
    global _NC_PROG, _NC_FLAGS
    b_h = np.asarray(inputs["b_h"], np.float32)
    b_out = np.asarray(inputs["b_out"], np.float32)
    flags = (bool(np.any(b_h[:HID])), bool(np.any(b_out)))
    if _NC_PROG is None or _NC_FLAGS != flags:
        _NC_PROG = build_nc(with_vbias=flags[0], with_obias=flags[1])
        _NC_FLAGS = flags
    in_maps = host_prep(inputs)
    res = run_bass_kernel_spmd(_NC_PROG, in_maps, list(range(NC)))
    y = np.stack([res.results[c]["y"] for c in range(NC)], axis=1)  # [4, NC, T, DIM]
    return np.ascontiguousarray(y.reshape(4, NSEQ, DIM)[None]).astype(np.float32)


# revision 9
# speedup vs baseline: 1.0917x; 1.0844x over previous
"""Trainium2 Bass kernel for nn_FLASH_40458591928592 (sparse_attention).

Sequence-sharded over 8 NeuronCores: 1024 tokens (= 4 groups of 256) per core.
Mixed precision, validated against a numpy e4m3 simulation (rel 1.66e-2 < 2e-2):
  qk GEMM : fully fp8 DoubleRow (x*0.25 stationary-free scales, W*4)
  v GEMM  : fully fp8 DoubleRow; v_h and tails stored fp8 (storage only)
  gate    : k-tiles 0-1 of 8 fp8 DR, rest bf16 (same PSUM, products at scale 1)
  y       : HID k-tiles 0-3 of 16 fp8 DR (og et 0-3 written fp8*0.25 by DVE)

Phase order keeps the PE dense and the HAM clock warm:
  dummy warmup matmuls through the DMA lead-in -> v -> qk(3,2,0,1) -> sim/attn
  -> lk transposes -> kv chains (AllGather per e-half fires ~60us) -> quad+conv
  (overlapping the collectives) -> lin -> gate+y per stream.
One LDWEIGHTS feeds 2-4 matmuls everywhere (ec/ch/th/nch pairing).
SBUF is tag-chained across serial phases (vh->ogb, wv8->woutb, qkT->gt,
S_full->og8, t_half->wg8, lk->wout8, S_offb->xr, tails->ysb).
"""

from contextlib import ExitStack

import numpy as np
import ml_dtypes

import concourse.tile as tile
from concourse import bacc, mybir
from concourse.bass_utils import run_bass_kernel_spmd
from concourse.masks import make_identity

BF = mybir.dt.bfloat16
F8 = mybir.dt.float8e4
F32 = mybir.dt.float32
bf16 = ml_dtypes.bfloat16
fp8 = ml_dtypes.float8_e4m3
DRM = mybir.MatmulPerfMode.DoubleRow

G = 256
DIM = 1024
HID = 2048
DQK = 128
NSEQ = 8192
NC = 8
T = NSEQ // NC        # 1024 tokens per core
NG = T // G           # 4 groups per core
KD = DIM // 128       # 8 k-tiles over dim
ET = HID // 128       # 16 e-tiles over hid
TT = T // 128         # 8 token tiles
EH = HID // 2         # 1024 cols per e-half

KG8 = 2               # gate fp8 k-tiles (of KD); one DR pair
KY8 = 4               # y fp8 k-tiles (of ET); must be even
KB = KD - KG8         # bf16 k-tiles for gate
XS = 0.25             # fp8 x-side scale
WS = 4.0              # fp8 w-side scale (XS*WS == 1 -> shared-PSUM)
NWARM = 90            # HAM warmup dummy matmuls

AF = mybir.ActivationFunctionType
ALU = mybir.AluOpType

DEBUG_DUMPS = False
WITH_VBIAS = True
WITH_OBIAS = True


def _emit(tc, ap):
    nc = tc.nc
    with ExitStack() as ctx:
        consts = ctx.enter_context(tc.tile_pool(name="consts", bufs=1))
        p_x8 = ctx.enter_context(tc.tile_pool(name="x8", bufs=2))
        p_xtb = ctx.enter_context(tc.tile_pool(name="xtb", bufs=3))
        p_x08 = ctx.enter_context(tc.tile_pool(name="x08", bufs=1))
        p_big = ctx.enter_context(tc.tile_pool(name="big", bufs=1))   # vh8 -> ogb
        p_qog = ctx.enter_context(tc.tile_pool(name="qog", bufs=1))   # qkT -> gt
        p_lk = ctx.enter_context(tc.tile_pool(name="lk", bufs=1))     # lk -> wout8
        p_w1 = ctx.enter_context(tc.tile_pool(name="w1", bufs=1))     # wv8 -> woutb
        p_w2 = ctx.enter_context(tc.tile_pool(name="w2", bufs=1))     # wgb
        p_tails = ctx.enter_context(tc.tile_pool(name="tails", bufs=1))  # -> ysb
        p_so = ctx.enter_context(tc.tile_pool(name="so", bufs=2))     # t_half -> wg8
        p_sob = ctx.enter_context(tc.tile_pool(name="sob", bufs=2))   # xr
        p_sf = ctx.enter_context(tc.tile_pool(name="sf", bufs=1))     # S_full -> og8
        p_tr = ctx.enter_context(tc.tile_pool(name="tr", bufs=1))
        p_a0 = ctx.enter_context(tc.tile_pool(name="a0", bufs=4))
        p_a1 = ctx.enter_context(tc.tile_pool(name="a1", bufs=4))
        p_outT = ctx.enter_context(tc.tile_pool(name="outT", bufs=1))
        ps1 = ctx.enter_context(tc.tile_pool(name="ps1", bufs=4, space="PSUM"))
        ps2 = ctx.enter_context(tc.tile_pool(name="ps2", bufs=2, space="PSUM"))
        pskv = ctx.enter_context(tc.tile_pool(name="pskv", bufs=2, space="PSUM"))

        # ---- HAM warmup: keep PE busy through the DMA lead-in ----
        ident = consts.tile([128, 128], BF, tag="ident")
        make_identity(nc, ident)
        for _ in range(NWARM):
            pw = ps2.tile([128, 128], F32, tag="ps2", name="pw")
            nc.tensor.matmul(pw, ident, ident, start=True, stop=True)

        # ---- first DMAs: v-GEMM inputs, then qk weights ----
        x08 = p_x08.tile([128, KD, T], F8, tag="x08")
        nc.sync.dma_start(x08, ap["xt08"].rearrange("(kt p) t -> p kt t", p=128))
        wv8 = p_w1.tile([128, KD, HID], F8, tag="w1")
        nc.scalar.dma_start(wv8, ap["wv8"].rearrange("(kt p) e -> p kt e", p=128))
        bqk = consts.tile([128, 1], F32, tag="bqk")
        nc.scalar.dma_start(bqk, ap["bqk"])
        wqk8 = consts.tile([128, KD, DQK], F8, tag="wqk8")
        nc.scalar.dma_start(wqk8, ap["wqk8"].rearrange("(kt p) q -> p kt q", p=128))
        xh8 = consts.tile([128, KD, 32], F8, tag="xh8")
        nc.scalar.dma_start(xh8, ap["xh8"].rearrange("(kt p) t -> p kt t", p=128))

        x8f = {0: x08}

        def load_x8f(s):
            t8 = p_x8.tile([128, KD, T], F8, tag="x8", name=f"x8_{s}")
            nc.sync.dma_start(t8, ap["xt8"][s - 1].rearrange("(kt p) t -> p kt t",
                                                             p=128))
            x8f[s] = t8

        load_x8f(3)
        load_x8f(2)

        # warm-up collective: absorbs first-collective setup latency
        cwarm = consts.tile([128, 16], BF, tag="cwarm")
        nc.vector.memset(cwarm, 0.0)
        nc.sync.dma_start(ap["cc_warm_in"], cwarm)
        nc.gpsimd.collective_compute(
            "AllGather", ALU.bypass, replica_groups=[list(range(NC))],
            ins=[ap["cc_warm_in"]], outs=[ap["cc_warm_out"]])

        # remaining consts (DMA behind weights on scalar queue)
        triu = consts.tile([128, 128], BF, tag="triu")
        nc.scalar.dma_start(triu, ap["triu"])
        bdiag = consts.tile([128, 128], BF, tag="bdiag")
        nc.scalar.dma_start(bdiag, ap["bdiag"])
        bcorn = consts.tile([128, 128], BF, tag="bcorn")
        nc.scalar.dma_start(bcorn, ap["bcorn"])
        bprev = consts.tile([32, 32], BF, tag="bprev")
        nc.scalar.dma_start(bprev, ap["bprev"])
        hmask = consts.tile([32, 1], F32, tag="hmask")
        nc.scalar.dma_start(hmask, ap["hmask"])
        wsumw = consts.tile([128, NC], F32, tag="wsumw")
        nc.scalar.dma_start(wsumw, ap["wsumw"])
        bgate = consts.tile([128, ET], F32, tag="bgate")
        nc.scalar.dma_start(bgate, ap["bgate"])
        if WITH_VBIAS or WITH_OBIAS:
            ones_t = consts.tile([1, 1024], BF, tag="ones")
            nc.vector.memset(ones_t, 1.0)
        if WITH_VBIAS:
            wvb = consts.tile([1, HID], BF, tag="wvb")
            nc.scalar.dma_start(wvb, ap["wvb"])
        if WITH_OBIAS:
            bout = consts.tile([1, DIM], BF, tag="bout")
            nc.scalar.dma_start(bout, ap["bout"])

        # ---- v GEMM: fp8 DR, one xt-pair LDWEIGHTS feeds 4 e-chunks ----
        v_h = p_big.tile([128, TT, HID], F8, tag="big", name="v_h")
        for tt in range(TT):
            pv = [ps1.tile([128, 512], F32, tag="ps1", name="pv") for _ in range(4)]
            for kp in range(KD // 2):
                for ec in range(4):
                    nc.tensor.matmul(
                        pv[ec], x08[:, 2 * kp:2 * kp + 2, tt * 128:(tt + 1) * 128],
                        wv8[:, 2 * kp:2 * kp + 2, ec * 512:(ec + 1) * 512],
                        start=(kp == 0),
                        stop=(kp == KD // 2 - 1 and not WITH_VBIAS),
                        perf_mode=DRM, skip_group_check=True)
            if WITH_VBIAS:
                for ec in range(4):
                    nc.tensor.matmul(pv[ec], ones_t[0:1, 0:128],
                                     wvb[0:1, ec * 512:(ec + 1) * 512],
                                     start=False, stop=True, skip_group_check=True)
            for ec in range(4):
                nc.scalar.activation(v_h[:, tt, ec * 512:(ec + 1) * 512], pv[ec],
                                     AF.Silu, bias=0.0, scale=1.0)

        # halo: last 32 tokens of the previous core (masked on core 0)
        tails = p_tails.tile([32, NG, HID], F8, tag="tails")
        ph = [ps1.tile([32, 512], F32, tag="ps1", name="ph") for _ in range(4)]
        for kp in range(KD // 2):
            for ec in range(4):
                nc.tensor.matmul(
                    ph[ec], xh8[:, 2 * kp:2 * kp + 2, :],
                    wv8[:, 2 * kp:2 * kp + 2, ec * 512:(ec + 1) * 512],
                    start=(kp == 0), stop=(kp == KD // 2 - 1 and not WITH_VBIAS),
                    perf_mode=DRM, skip_group_check=True)
        if WITH_VBIAS:
            for ec in range(4):
                nc.tensor.matmul(ph[ec], ones_t[0:1, 0:32],
                                 wvb[0:1, ec * 512:(ec + 1) * 512],
                                 start=False, stop=True, skip_group_check=True)
        for ec in range(4):
            nc.scalar.activation(tails[:, 0, ec * 512:(ec + 1) * 512], ph[ec],
                                 AF.Silu, bias=0.0, scale=1.0)
            nc.vector.tensor_scalar_mul(tails[:, 0, ec * 512:(ec + 1) * 512],
                                        tails[:, 0, ec * 512:(ec + 1) * 512], hmask)
        for g in range(1, NG):
            nc.scalar.dma_start(tails[:, g, :], v_h[96:128, 2 * g - 1, :])

        # ---- qk streams: fully fp8 DR, ch-paired ----
        qkT = p_qog.tile([128, 4, T], BF, tag="qog", name="qkT")
        for si, s in enumerate((3, 2, 0, 1)):
            pc = [ps1.tile([128, 512], F32, tag="ps1", name="pc") for _ in range(2)]
            for kp in range(KD // 2):
                for ch in range(2):
                    nc.tensor.matmul(pc[ch], wqk8[:, 2 * kp:2 * kp + 2, :],
                                     x8f[s][:, 2 * kp:2 * kp + 2,
                                            ch * 512:(ch + 1) * 512],
                                     start=(kp == 0), stop=(kp == KD // 2 - 1),
                                     perf_mode=DRM, skip_group_check=True)
            for ch in range(2):
                nc.scalar.activation(qkT[:, s, ch * 512:(ch + 1) * 512], pc[ch],
                                     AF.Silu, bias=bqk, scale=1.0)
            if si == 0:
                load_x8f(1)   # slot rotation WARs on qk3's reads

        # ---- sim/attn per group (conv band folded into bdiag/bcorn) ----
        attn0, attn1 = [], []
        for g in range(NG):
            i0 = g * G
            a0 = p_a0.tile([128, 256], BF, tag="a0")
            ps = ps2.tile([128, 256], F32, tag="ps2")
            nc.tensor.matmul(ps, qkT[:, 2, i0:i0 + 128], qkT[:, 0, i0:i0 + 256],
                             start=True, stop=True)
            nc.scalar.activation(a0, ps, AF.Relu, bias=0.0, scale=1.0 / G)
            nc.vector.tensor_mul(a0[:, 0:128], a0[:, 0:128], triu)
            nc.vector.tensor_mul(a0, a0, a0)
            nc.vector.tensor_add(a0[:, 0:128], a0[:, 0:128], bdiag)
            nc.vector.tensor_add(a0[:, 128:256], a0[:, 128:256], bcorn)
            attn0.append(a0)

            a1 = p_a1.tile([128, 128], BF, tag="a1")
            ps = ps2.tile([128, 256], F32, tag="ps2")
            nc.tensor.matmul(ps[:, 0:128], qkT[:, 2, i0 + 128:i0 + 256],
                             qkT[:, 0, i0 + 128:i0 + 256], start=True, stop=True)
            nc.scalar.activation(a1, ps[:, 0:128], AF.Relu, bias=0.0, scale=1.0 / G)
            nc.vector.tensor_mul(a1, a1, triu)
            nc.vector.tensor_mul(a1, a1, a1)
            nc.vector.tensor_add(a1, a1, bdiag)
            attn1.append(a1)

        # lk (stream 3) token-major via PE transpose
        lk_tok = p_lk.tile([128, TT, 128], BF, tag="lk", name="lk_tok")
        for tt in range(TT):
            pt = ps2.tile([128, 128], BF, tag="ps2", name="pt")
            nc.tensor.transpose(pt, qkT[:, 3, tt * 128:(tt + 1) * 128], ident)
            nc.vector.tensor_copy(lk_tok[:, tt, :], pt)

        # ---- kv chains + AllGather per e-half ----
        S_full = p_sf.tile([128, NG, HID], BF, tag="sf", name="S_full")

        def wsum_half(eh, t_half):
            e0 = eh * EH
            cc_in, cc_out = ap[f"cc_in{eh}"], ap[f"cc_out{eh}"]
            nc.scalar.dma_start(cc_in, t_half)
            nc.gpsimd.collective_compute(
                "AllGather", ALU.bypass, replica_groups=[list(range(NC))],
                ins=[cc_in], outs=[cc_out])
            s0 = S_full[:, 0, e0:e0 + EH]
            for r in range(NC):
                tr = p_tr.tile([128, EH], BF, tag="tr")
                nc.sync.dma_start(tr, cc_out[r * 128:(r + 1) * 128, :])
                if r == 0:
                    nc.vector.tensor_scalar_mul(s0, tr, wsumw[:, 0:1])
                else:
                    nc.vector.scalar_tensor_tensor(
                        s0, tr, wsumw[:, r:r + 1], s0, op0=ALU.mult, op1=ALU.add)
            for g in range(1, NG):
                nc.vector.tensor_add(S_full[:, g, e0:e0 + EH],
                                     S_full[:, g, e0:e0 + EH], s0)

        for eh in range(2):
            e0 = eh * EH
            t_half = p_so.tile([128, EH], BF, tag="so", name="t_half")
            pk = [pskv.tile([128, 512], F32, tag="pskv", name="pk")
                  for _ in range(2)]
            for g in range(NG):
                for jt in range(2):
                    for ec in range(2):
                        nc.tensor.matmul(
                            pk[ec], lk_tok[:, 2 * g + jt, :],
                            v_h[:, 2 * g + jt, e0 + ec * 512:e0 + (ec + 1) * 512],
                            start=(g == 0 and jt == 0),
                            stop=(g == NG - 1 and jt == 1),
                            skip_group_check=True)
                for ec in range(2):
                    dst = (S_full[:, g + 1, e0 + ec * 512:e0 + (ec + 1) * 512]
                           if g < NG - 1 else t_half[:, ec * 512:(ec + 1) * 512])
                    nc.scalar.activation(dst, pk[ec], AF.Copy, bias=0.0,
                                         scale=1.0 / G)
            wsum_half(eh, t_half)

        # ---- quad + conv boundary -> outT ----
        outT = p_outT.tile([128, ET, T], BF, tag="outT")
        for eh in range(2):
            e0 = eh * EH
            for g in range(NG):
                for et in range(8):
                    ec0 = e0 + et * 128
                    po = ps2.tile([128, 256], F32, tag="ps2", name="po")
                    nc.tensor.matmul(po, v_h[:, 2 * g, ec0:ec0 + 128], attn0[g],
                                     start=True, stop=False, skip_group_check=True)
                    nc.tensor.matmul(po[:, 128:256],
                                     v_h[:, 2 * g + 1, ec0:ec0 + 128],
                                     attn1[g], start=False, stop=False,
                                     skip_group_check=True)
                    nc.tensor.matmul(po[:, 0:32], tails[:, g, ec0:ec0 + 128], bprev,
                                     start=False, stop=True, skip_group_check=True)
                    nc.scalar.activation(outT[:, eh * 8 + et, g * G:(g + 1) * G],
                                         po, AF.Copy, bias=0.0, scale=1.0)

        # gate weights (DMA while PE chews on quad)
        wgb = p_w2.tile([128, KB, HID], BF, tag="w2", name="wgb")
        for kt in range(KB):
            nc.scalar.dma_start(wgb[:, kt, :],
                                ap["wgb"][kt * 128:(kt + 1) * 128, :])
        wg8 = p_so.tile([128, KG8, HID], F8, tag="so", name="wg8")
        nc.scalar.dma_start(wg8, ap["wg8"].rearrange("(kt p) e -> p kt e", p=128))

        # scheduler fence: without it the scheduler hoists the lin matmuls
        # (which wait on the AllGather+wsum) ahead of quad and parks the PE
        tc.no_sync_barrier()

        # ---- lin joined via DVE add into outT ----
        for eh in range(2):
            for g in range(NG):
                for et in range(eh * 8, eh * 8 + 8):
                    po = ps2.tile([128, 256], F32, tag="ps2", name="po")
                    nc.tensor.matmul(po, S_full[:, g, et * 128:(et + 1) * 128],
                                     qkT[:, 1, g * G:(g + 1) * G],
                                     start=True, stop=True)
                    nc.vector.tensor_add(outT[:, et, g * G:(g + 1) * G],
                                         outT[:, et, g * G:(g + 1) * G], po)

        # out-projection weights (DMA during lin/first gate)
        woutb = p_w1.tile([128, ET - KY8, DIM], BF, tag="w1", name="woutb")
        for kt in range(ET - KY8):
            nc.scalar.dma_start(woutb[:, kt, :],
                                ap["woutb"][kt * 128:(kt + 1) * 128, :])
        wout8 = p_lk.tile([128, KY8, DIM], F8, tag="lk", name="wout8")
        nc.scalar.dma_start(wout8, ap["wout8"].rearrange("(kt p) n -> p kt n",
                                                         p=128))

        if DEBUG_DUMPS:
            nc.sync.dma_start(ap["dbg_qkT"], qkT)
            nc.sync.dma_start(ap["dbg_vh"], v_h)
            nc.sync.dma_start(ap["dbg_outT"], outT)
            nc.sync.dma_start(ap["dbg_sfull"], S_full)

        # bf16 gate inputs (k-tiles 2..7), loaded during lin / earlier streams
        xtb = {}

        def load_xtb(s):
            halves = []
            for q in range(2):
                h = p_xtb.tile([128, KB // 2, T], BF, tag="xtb", name=f"xtb{s}_{q}")
                nc.sync.dma_start(
                    h, ap["xtb"][s, q * 384:(q + 1) * 384, :].rearrange(
                        "(kt p) t -> p kt t", p=128))
                halves.append(h)
            xtb[s] = halves

        load_xtb(0)
        load_xtb(1)

        # ---- gate + y per stream (th-paired gate, nch-paired y) ----
        for s in range(4):
            og8 = p_sf.tile([128, KY8, T], F8, tag="sf", name="og8")
            ogb = p_big.tile([128, ET - KY8, T], BF, tag="big", name="ogb")
            for et in range(ET):
                pg = [ps1.tile([128, 512], F32, tag="ps1", name="pg")
                      for _ in range(2)]
                for th in range(2):
                    nc.tensor.matmul(
                        pg[th], wg8[:, 0:KG8, et * 128:(et + 1) * 128],
                        x8f[s][:, 0:KG8, th * 512:(th + 1) * 512],
                        start=True, stop=False,
                        perf_mode=DRM, skip_group_check=True)
                for kt in range(KB):
                    xs_t = xtb[s][kt // 3][:, kt % 3, :]
                    for th in range(2):
                        nc.tensor.matmul(
                            pg[th], wgb[:, kt, et * 128:(et + 1) * 128],
                            xs_t[:, th * 512:(th + 1) * 512],
                            start=False, stop=(kt == KB - 1),
                            skip_group_check=True)
                for th in range(2):
                    t0 = th * 512
                    if et < KY8:
                        gt = p_qog.tile([128, 512], BF, tag="qog", name="gt")
                        nc.scalar.activation(gt, pg[th], AF.Silu,
                                             bias=bgate[:, et:et + 1], scale=1.0)
                        nc.vector.scalar_tensor_tensor(
                            og8[:, et, t0:t0 + 512], gt, XS,
                            outT[:, et, t0:t0 + 512], op0=ALU.mult, op1=ALU.mult)
                    else:
                        eb = et - KY8
                        nc.scalar.activation(ogb[:, eb, t0:t0 + 512], pg[th],
                                             AF.Silu, bias=bgate[:, et:et + 1],
                                             scale=1.0)
                        nc.vector.tensor_mul(ogb[:, eb, t0:t0 + 512],
                                             ogb[:, eb, t0:t0 + 512],
                                             outT[:, et, t0:t0 + 512])
            if s < 2:
                load_xtb(s + 2)
            if s == 0:
                load_x8f(1)
            elif s == 1:
                load_x8f(2)
            elif s == 2:
                load_x8f(3)
            for tl in range(TT):
                tsl = slice(tl * 128, (tl + 1) * 128)
                pn = [ps1.tile([128, 512], F32, tag="ps1", name="pn")
                      for _ in range(2)]
                for kp in range(KY8 // 2):
                    for nch in range(2):
                        nc.tensor.matmul(
                            pn[nch], og8[:, 2 * kp:2 * kp + 2, tsl],
                            wout8[:, 2 * kp:2 * kp + 2,
                                  nch * 512:(nch + 1) * 512],
                            start=(kp == 0), stop=False,
                            perf_mode=DRM, skip_group_check=True)
                for kt in range(ET - KY8):
                    for nch in range(2):
                        nc.tensor.matmul(
                            pn[nch], ogb[:, kt, tsl],
                            woutb[:, kt, nch * 512:(nch + 1) * 512],
                            start=False,
                            stop=(kt == ET - KY8 - 1 and not WITH_OBIAS),
                            skip_group_check=True)
                if WITH_OBIAS:
                    for nch in range(2):
                        nc.tensor.matmul(pn[nch], ones_t[0:1, 0:128],
                                         bout[0:1, nch * 512:(nch + 1) * 512],
                                         start=False, stop=True,
                                         skip_group_check=True)
                for nch in range(2):
                    n0 = nch * 512
                    xr = p_sob.tile([128, 512], F32, tag="xr", name="xr")
                    nc.sync.dma_start(xr, ap["xtok"][s, tsl, n0:n0 + 512])
                    ysb = p_tails.tile([128, 512], F32, tag="ysb", name="ysb",
                                       bufs=2)
                    nc.vector.scalar_tensor_tensor(
                        ysb, pn[nch], 0.0, xr, op0=ALU.add, op1=ALU.add)
                    nc.sync.dma_start(ap["y"][s, tsl, n0:n0 + 512], ysb)


def build_nc(with_vbias=None, with_obias=None):
    global WITH_VBIAS, WITH_OBIAS
    if with_vbias is not None:
        WITH_VBIAS = with_vbias
    if with_obias is not None:
        WITH_OBIAS = with_obias
    nc = bacc.Bacc("TRN2", target_bir_lowering=False, debug=False, num_devices=NC)
    ap = {}

    def dram(name, shape, dt, kind=None, addr_space=None):
        kw = {}
        if kind:
            kw["kind"] = kind
        if addr_space:
            kw["addr_space"] = addr_space
        ap[name] = nc.dram_tensor(name, shape, dt, **kw).ap()

    dram("xtb", [4, KB * 128, T], BF, kind="ExternalInput")
    dram("xt8", [3, DIM, T], F8, kind="ExternalInput")
    dram("xt08", [DIM, T], F8, kind="ExternalInput")
    dram("xh8", [DIM, 32], F8, kind="ExternalInput")
    dram("xtok", [4, T, DIM], F32, kind="ExternalInput")
    dram("wv8", [DIM, HID], F8, kind="ExternalInput")
    dram("wgb", [KB * 128, HID], BF, kind="ExternalInput")
    dram("wg8", [KG8 * 128, HID], F8, kind="ExternalInput")
    dram("wqk8", [DIM, DQK], F8, kind="ExternalInput")
    dram("woutb", [(ET - KY8) * 128, DIM], BF, kind="ExternalInput")
    dram("wout8", [KY8 * 128, DIM], F8, kind="ExternalInput")
    dram("wvb", [1, HID], BF, kind="ExternalInput")
    dram("bout", [1, DIM], BF, kind="ExternalInput")
    dram("bgate", [128, ET], F32, kind="ExternalInput")
    dram("bqk", [128, 1], F32, kind="ExternalInput")
    dram("triu", [128, 128], BF, kind="ExternalInput")
    dram("bdiag", [128, 128], BF, kind="ExternalInput")
    dram("bcorn", [128, 128], BF, kind="ExternalInput")
    dram("bprev", [32, 32], BF, kind="ExternalInput")
    dram("hmask", [32, 1], F32, kind="ExternalInput")
    dram("wsumw", [128, NC], F32, kind="ExternalInput")
    if DEBUG_DUMPS:
        dram("dbg_qkT", [128, 4, T], BF, kind="ExternalOutput")
        dram("dbg_vh", [128, TT, HID], F8, kind="ExternalOutput")
        dram("dbg_outT", [128, ET, T], BF, kind="ExternalOutput")
        dram("dbg_sfull", [128, NG, HID], BF, kind="ExternalOutput")
    dram("cc_warm_in", [128, 16], BF)
    dram("cc_warm_out", [NC * 128, 16], BF, addr_space="Shared")
    dram("cc_in0", [128, EH], BF)
    dram("cc_out0", [NC * 128, EH], BF, addr_space="Shared")
    dram("cc_in1", [128, EH], BF)
    dram("cc_out1", [NC * 128, EH], BF, addr_space="Shared")
    dram("y", [4, T, DIM], F32, kind="ExternalOutput")

    with tile.TileContext(nc) as tc:
        _emit(tc, ap)
    nc.compile()
    return nc


def _f8(a):
    return np.clip(a, -240.0, 240.0).astype(fp8)


def host_prep(inputs):
    """Pure layout transforms: shard, transpose, cast, build conv-band consts."""
    x = np.ascontiguousarray(np.asarray(inputs["x"], np.float32)[0])  # [4, N, DIM]
    W_h = np.asarray(inputs["W_h"], np.float32)
    b_h = np.asarray(inputs["b_h"], np.float32)
    W_qk = np.asarray(inputs["W_qk"], np.float32)
    b_qk = np.asarray(inputs["b_qk"], np.float32)
    W_out = np.asarray(inputs["W_out"], np.float32)
    b_out = np.asarray(inputs["b_out"], np.float32)
    cw = np.asarray(inputs["conv_w"], np.float32)

    jj = np.arange(128)[:, None]
    ii = np.arange(128)[None, :]
    d = ii - jj
    triu = (ii >= jj).astype(bf16)
    bdiag = np.where((d >= 0) & (d <= 31), cw[np.clip(31 - d, 0, 62)], 0.0).astype(bf16)
    dc = (ii + 128) - jj
    bcorn = np.where((dc >= 0) & (dc <= 31),
                     cw[np.clip(31 - dc, 0, 62)], 0.0).astype(bf16)
    jt = np.arange(32)[:, None]
    ip = np.arange(32)[None, :]
    dp = ip + 32 - jt
    bprev = np.where((dp >= 1) & (dp <= 31),
                     cw[np.clip(31 - dp, 0, 62)], 0.0).astype(bf16)

    kb0 = KG8 * 128
    ky0 = KY8 * 128
    common = {
        "wv8": _f8(W_h[:, :HID] * WS),
        "wgb": np.ascontiguousarray(W_h[kb0:, HID:]).astype(bf16),
        "wg8": _f8(W_h[:kb0, HID:] * WS),
        "wqk8": _f8(W_qk * WS),
        "woutb": np.ascontiguousarray(W_out[ky0:, :]).astype(bf16),
        "wout8": _f8(W_out[:ky0, :] * WS),
        "wvb": b_h[None, :HID].astype(bf16),
        "bout": b_out[None, :].astype(bf16),
        "bgate": np.ascontiguousarray(b_h[HID:].reshape(ET, 128).T).astype(np.float32),
        "bqk": b_qk[:, None].astype(np.float32),
        "triu": triu, "bdiag": bdiag, "bcorn": bcorn, "bprev": bprev,
    }

    in_maps = []
    for c in range(NC):
        sl = slice(c * T, (c + 1) * T)
        x_c = x[:, sl, :]
        xtb = np.zeros((4, KB * 128, T), bf16)
        xt8 = np.zeros((3, DIM, T), fp8)
        for s in range(4):
            xT = x_c[s].T
            xtb[s] = xT[kb0:].astype(bf16)
            if s > 0:
                xt8[s - 1] = _f8(xT * XS)
        xt08 = _f8(x_c[0].T * XS)
        if c > 0:
            xh8 = _f8(np.ascontiguousarray(x[0, c * T - 32:c * T, :].T) * XS)
        else:
            xh8 = np.zeros((DIM, 32), fp8)
        m = dict(common)
        m["xtb"] = xtb
        m["xt8"] = xt8
        m["xt08"] = xt08
        m["xh8"] = xh8
        m["xtok"] = np.ascontiguousarray(x_c)
        m["hmask"] = np.full((32, 1), 1.0 if c > 0 else 0.0, np.float32)
        w = np.zeros((128, NC), np.float32)
        w[:, :c] = 1.0
        m["wsumw"] = w
        in_maps.append(m)
    return in_maps


_NC_PROG = None
_NC_FLAGS = None


def kernel(**inputs):
    global _NC_PROG, _NC_FLAGS
    b_h = np.asarray(inputs["b_h"], np.float32)
    b_out = np.asarray(inputs["b_out"], np.float32)
    flags = (bool(np.any(b_h[:HID])), bool(np.any(b_out)))
    if _NC_PROG is None or _NC_FLAGS != flags:
        _NC_PROG = build_nc(with_vbias=flags[0], with_obias=flags[1])
        _NC_FLAGS = flags
    in_maps = host_prep(inputs)
    res = run_bass_kernel_spmd(_NC_PROG, in_maps, list(range(NC)))
    y = np.stack([res.results[c]["y"] for c in range(NC)], axis=1)  # [4, NC, T, DIM]
    return np.ascontiguousarray(y.reshape(4, NSEQ, DIM)[None]).astype(np.float32)


# revision 12
# speedup vs baseline: 1.1913x; 1.0913x over previous
"""Trainium2 Bass kernel for nn_FLASH_40458591928592 (sparse_attention).

Sequence-sharded over 8 NeuronCores: 1024 tokens (= 4 groups of 256) per core.
Mixed precision, validated against a numpy e4m3 simulation (rel 1.66e-2 < 2e-2):
  qk GEMM : fully fp8 DoubleRow (x*0.25 stationary-free scales, W*4)
  v GEMM  : fully fp8 DoubleRow; v_h and tails stored fp8 (storage only)
  gate    : k-tiles 0-1 of 8 fp8 DR, rest bf16 (same PSUM, products at scale 1)
  y       : HID k-tiles 0-3 of 16 fp8 DR (og et 0-3 written fp8*0.25 by DVE)

Phase order keeps the PE dense and the HAM clock warm:
  dummy warmup matmuls through the DMA lead-in -> v -> qk(3,2,0,1) -> sim/attn
  -> lk transposes -> kv chains (AllGather per e-half fires ~60us) -> quad+conv
  (overlapping the collectives) -> lin -> gate+y per stream.
One LDWEIGHTS feeds 2-4 matmuls everywhere (ec/ch/th/nch pairing).
SBUF is tag-chained across serial phases (vh->ogb, wv8->woutb, qkT->gt,
S_full->og8, t_half->wg8, lk->wout8, S_offb->xr, tails->ysb).
"""

from contextlib import ExitStack

import numpy as np
import ml_dtypes

import concourse.tile as tile
from concourse import bacc, mybir
from concourse.bass_utils import run_bass_kernel_spmd
from concourse.masks import make_identity

BF = mybir.dt.bfloat16
F8 = mybir.dt.float8e4
F32 = mybir.dt.float32
bf16 = ml_dtypes.bfloat16
fp8 = ml_dtypes.float8_e4m3
DRM = mybir.MatmulPerfMode.DoubleRow

G = 256
DIM = 1024
HID = 2048
DQK = 128
NSEQ = 8192
NC = 8
T = NSEQ // NC        # 1024 tokens per core
NG = T // G           # 4 groups per core
KD = DIM // 128       # 8 k-tiles over dim
ET = HID // 128       # 16 e-tiles over hid
TT = T // 128         # 8 token tiles
EH = HID // 2         # 1024 cols per e-half

KG8 = 2               # gate fp8 k-tiles (of KD); one DR pair
KY8 = 4               # y fp8 k-tiles (of ET); must be even
KB = KD - KG8         # bf16 k-tiles for gate
XS = 0.25             # fp8 x-side scale
WS = 4.0              # fp8 w-side scale (XS*WS == 1 -> shared-PSUM)
NWARM = 90            # HAM warmup dummy matmuls

AF = mybir.ActivationFunctionType
ALU = mybir.AluOpType

DEBUG_DUMPS = False
WITH_VBIAS = True
WITH_OBIAS = True


def _emit(tc, ap):
    nc = tc.nc
    with ExitStack() as ctx:
        consts = ctx.enter_context(tc.tile_pool(name="consts", bufs=1))
        p_x8 = ctx.enter_context(tc.tile_pool(name="x8", bufs=2))
        p_xtb = ctx.enter_context(tc.tile_pool(name="xtb", bufs=3))
        p_x08 = ctx.enter_context(tc.tile_pool(name="x08", bufs=1))
        p_big = ctx.enter_context(tc.tile_pool(name="big", bufs=1))   # vh8 -> ogb
        p_qog = ctx.enter_context(tc.tile_pool(name="qog", bufs=1))   # qkT -> gt
        p_lk = ctx.enter_context(tc.tile_pool(name="lk", bufs=1))     # lk -> wout8
        p_w1 = ctx.enter_context(tc.tile_pool(name="w1", bufs=1))     # wv8 -> woutb
        p_w2 = ctx.enter_context(tc.tile_pool(name="w2", bufs=1))     # wgb
        p_tails = ctx.enter_context(tc.tile_pool(name="tails", bufs=1))  # -> ysb
        p_so = ctx.enter_context(tc.tile_pool(name="so", bufs=2))     # t_half -> wg8
        p_sob = ctx.enter_context(tc.tile_pool(name="sob", bufs=2))   # xr
        p_sf = ctx.enter_context(tc.tile_pool(name="sf", bufs=1))     # S_full -> og8
        p_tr = ctx.enter_context(tc.tile_pool(name="tr", bufs=4))
        p_a0 = ctx.enter_context(tc.tile_pool(name="a0", bufs=4))
        p_a1 = ctx.enter_context(tc.tile_pool(name="a1", bufs=4))
        p_outT = ctx.enter_context(tc.tile_pool(name="outT", bufs=1))
        ps1 = ctx.enter_context(tc.tile_pool(name="ps1", bufs=4, space="PSUM"))
        ps2 = ctx.enter_context(tc.tile_pool(name="ps2", bufs=4, space="PSUM"))

        # warm-up collective first: its ~40us setup runs in the DMA shadow
        cwarm = consts.tile([128, 16], BF, tag="cwarm")
        nc.vector.memset(cwarm, 0.0)
        nc.sync.dma_start(ap["cc_warm_in"], cwarm)
        nc.gpsimd.collective_compute(
            "AllGather", ALU.bypass, replica_groups=[list(range(NC))],
            ins=[ap["cc_warm_in"]], outs=[ap["cc_warm_out"]])

        # ---- HAM warmup: keep PE busy through the DMA lead-in ----
        ident = consts.tile([128, 128], BF, tag="ident")
        make_identity(nc, ident)
        for _ in range(NWARM):
            pw = ps2.tile([128, 128], F32, tag="ps2", name="pw")
            nc.tensor.matmul(pw, ident, ident, start=True, stop=True)

        # ---- first DMAs: v-GEMM inputs, then qk weights ----
        x08 = p_x08.tile([128, KD, T], F8, tag="x08")
        nc.sync.dma_start(x08, ap["xt08"].rearrange("(kt p) t -> p kt t", p=128))
        wv8 = p_w1.tile([128, KD, HID], F8, tag="w1")
        nc.scalar.dma_start(wv8, ap["wv8"].rearrange("(kt p) e -> p kt e", p=128))
        bqk = consts.tile([128, 1], F32, tag="bqk")
        nc.scalar.dma_start(bqk, ap["bqk"])
        wqk8 = consts.tile([128, KD, DQK], F8, tag="wqk8")
        nc.scalar.dma_start(wqk8, ap["wqk8"].rearrange("(kt p) q -> p kt q", p=128))
        xh8 = consts.tile([128, KD, 32], F8, tag="xh8")
        nc.scalar.dma_start(xh8, ap["xh8"].rearrange("(kt p) t -> p kt t", p=128))

        x8f = {0: x08}

        def load_x8f(s):
            t8 = p_x8.tile([128, KD, T], F8, tag="x8", name=f"x8_{s}")
            nc.sync.dma_start(t8, ap["xt8"][s - 1].rearrange("(kt p) t -> p kt t",
                                                             p=128))
            x8f[s] = t8

        load_x8f(3)
        load_x8f(2)

        # remaining consts (DMA behind weights on scalar queue)
        triu = consts.tile([128, 128], BF, tag="triu")
        nc.scalar.dma_start(triu, ap["triu"])
        bdiag = consts.tile([128, 128], BF, tag="bdiag")
        nc.scalar.dma_start(bdiag, ap["bdiag"])
        bcorn = consts.tile([128, 128], BF, tag="bcorn")
        nc.scalar.dma_start(bcorn, ap["bcorn"])
        bprev = consts.tile([32, 32], BF, tag="bprev")
        nc.scalar.dma_start(bprev, ap["bprev"])
        hmask = consts.tile([32, 1], F32, tag="hmask")
        nc.scalar.dma_start(hmask, ap["hmask"])
        wsumw = consts.tile([128, NC], F32, tag="wsumw")
        nc.scalar.dma_start(wsumw, ap["wsumw"])
        bgate = consts.tile([128, ET], F32, tag="bgate")
        nc.scalar.dma_start(bgate, ap["bgate"])
        if WITH_VBIAS or WITH_OBIAS:
            ones_t = consts.tile([1, 1024], BF, tag="ones")
            nc.vector.memset(ones_t, 1.0)
        if WITH_VBIAS:
            wvb = consts.tile([1, HID], BF, tag="wvb")
            nc.scalar.dma_start(wvb, ap["wvb"])
        if WITH_OBIAS:
            bout = consts.tile([1, DIM], BF, tag="bout")
            nc.scalar.dma_start(bout, ap["bout"])

        # ---- v GEMM: fp8 DR, one xt-pair LDWEIGHTS feeds 4 e-chunks ----
        v_h = p_big.tile([128, TT, HID], F8, tag="big", name="v_h")
        for tt in range(TT):
            pv = [ps1.tile([128, 512], F32, tag="ps1", name="pv") for _ in range(4)]
            for kp in range(KD // 2):
                for ec in range(4):
                    nc.tensor.matmul(
                        pv[ec], x08[:, 2 * kp:2 * kp + 2, tt * 128:(tt + 1) * 128],
                        wv8[:, 2 * kp:2 * kp + 2, ec * 512:(ec + 1) * 512],
                        start=(kp == 0),
                        stop=(kp == KD // 2 - 1 and not WITH_VBIAS),
                        perf_mode=DRM, skip_group_check=True)
            if WITH_VBIAS:
                for ec in range(4):
                    nc.tensor.matmul(pv[ec], ones_t[0:1, 0:128],
                                     wvb[0:1, ec * 512:(ec + 1) * 512],
                                     start=False, stop=True, skip_group_check=True)
            for ec in range(4):
                nc.scalar.activation(v_h[:, tt, ec * 512:(ec + 1) * 512], pv[ec],
                                     AF.Silu, bias=0.0, scale=1.0)

        # halo: last 32 tokens of the previous core (masked on core 0)
        tails = p_tails.tile([32, NG, HID], F8, tag="tails")
        ph = [ps1.tile([32, 512], F32, tag="ps1", name="ph") for _ in range(4)]
        for kp in range(KD // 2):
            for ec in range(4):
                nc.tensor.matmul(
                    ph[ec], xh8[:, 2 * kp:2 * kp + 2, :],
                    wv8[:, 2 * kp:2 * kp + 2, ec * 512:(ec + 1) * 512],
                    start=(kp == 0), stop=(kp == KD // 2 - 1 and not WITH_VBIAS),
                    perf_mode=DRM, skip_group_check=True)
        if WITH_VBIAS:
            for ec in range(4):
                nc.tensor.matmul(ph[ec], ones_t[0:1, 0:32],
                                 wvb[0:1, ec * 512:(ec + 1) * 512],
                                 start=False, stop=True, skip_group_check=True)
        for ec in range(4):
            nc.scalar.activation(tails[:, 0, ec * 512:(ec + 1) * 512], ph[ec],
                                 AF.Silu, bias=0.0, scale=1.0)
            nc.vector.tensor_scalar_mul(tails[:, 0, ec * 512:(ec + 1) * 512],
                                        tails[:, 0, ec * 512:(ec + 1) * 512], hmask)
        for g in range(1, NG):
            nc.scalar.dma_start(tails[:, g, :], v_h[96:128, 2 * g - 1, :])

        # ---- qk streams: fully fp8 DR, ch-paired ----
        qkT = p_qog.tile([128, 4, T], BF, tag="qog", name="qkT")

        def qk_stream(s):
            pc = [ps1.tile([128, 512], F32, tag="ps1", name="pc") for _ in range(2)]
            for kp in range(KD // 2):
                for ch in range(2):
                    nc.tensor.matmul(pc[ch], wqk8[:, 2 * kp:2 * kp + 2, :],
                                     x8f[s][:, 2 * kp:2 * kp + 2,
                                            ch * 512:(ch + 1) * 512],
                                     start=(kp == 0), stop=(kp == KD // 2 - 1),
                                     perf_mode=DRM, skip_group_check=True)
            for ch in range(2):
                nc.scalar.activation(qkT[:, s, ch * 512:(ch + 1) * 512], pc[ch],
                                     AF.Silu, bias=bqk, scale=1.0)

        qk_stream(3)
        load_x8f(1)   # slot rotation WARs on qk3's reads

        # lk (stream 3) token-major via PE transpose
        lk_tok = p_lk.tile([128, TT, 128], BF, tag="lk", name="lk_tok")
        for tt in range(TT):
            pt = ps2.tile([128, 128], BF, tag="ps2", name="pt")
            nc.tensor.transpose(pt, qkT[:, 3, tt * 128:(tt + 1) * 128], ident)
            nc.vector.tensor_copy(lk_tok[:, tt, :], pt)

        # ---- kv chains + AllGather per e-half ----
        S_full = p_sf.tile([128, NG, HID], BF, tag="sf", name="S_full")

        def wsum_half(eh, t_half):
            e0 = eh * EH
            cc_in, cc_out = ap[f"cc_in{eh}"], ap[f"cc_out{eh}"]
            nc.scalar.dma_start(cc_in, t_half)
            nc.gpsimd.collective_compute(
                "AllGather", ALU.bypass, replica_groups=[list(range(NC))],
                ins=[cc_in], outs=[cc_out])
            for r in range(NC):
                for hh in range(2):
                    s0 = S_full[:, 0, e0 + hh * 512:e0 + (hh + 1) * 512]
                    tr = p_tr.tile([128, 512], BF, tag="tr")
                    nc.sync.dma_start(
                        tr, cc_out[r * 128:(r + 1) * 128,
                                   hh * 512:(hh + 1) * 512])
                    if r == 0:
                        nc.vector.tensor_scalar_mul(s0, tr, wsumw[:, 0:1])
                    else:
                        nc.vector.scalar_tensor_tensor(
                            s0, tr, wsumw[:, r:r + 1], s0,
                            op0=ALU.mult, op1=ALU.add)
            for g in range(1, NG):
                nc.vector.tensor_add(S_full[:, g, e0:e0 + EH],
                                     S_full[:, g, e0:e0 + EH],
                                     S_full[:, 0, e0:e0 + EH])

        for eh in range(2):
            e0 = eh * EH
            t_half = p_so.tile([128, EH], BF, tag="so", name="t_half")
            for g in range(NG):
                pk = [ps1.tile([128, 512], F32, tag="ps1", name="pk")
                      for _ in range(2)]
                for jt in range(2):
                    for ec in range(2):
                        nc.tensor.matmul(
                            pk[ec], lk_tok[:, 2 * g + jt, :],
                            v_h[:, 2 * g + jt, e0 + ec * 512:e0 + (ec + 1) * 512],
                            start=(jt == 0), stop=(jt == 1),
                            skip_group_check=True)
                for ec in range(2):
                    dst = (S_full[:, g + 1, e0 + ec * 512:e0 + (ec + 1) * 512]
                           if g < NG - 1 else t_half[:, ec * 512:(ec + 1) * 512])
                    nc.scalar.activation(dst, pk[ec], AF.Copy, bias=0.0,
                                         scale=1.0 / G)
            # exclusive-prefix over local groups on DVE (off the PE path)
            for g in range(2, NG):
                nc.vector.tensor_add(S_full[:, g, e0:e0 + EH],
                                     S_full[:, g, e0:e0 + EH],
                                     S_full[:, g - 1, e0:e0 + EH])
            nc.vector.tensor_add(t_half, t_half, S_full[:, NG - 1, e0:e0 + EH])
            wsum_half(eh, t_half)

        for s in (2, 0, 1):
            qk_stream(s)

        # ---- sim/attn per group (conv band folded into bdiag/bcorn) ----
        attn0, attn1 = [], []
        for g in range(NG):
            i0 = g * G
            a0 = p_a0.tile([128, 256], BF, tag="a0")
            ps = ps2.tile([128, 256], F32, tag="ps2")
            nc.tensor.matmul(ps, qkT[:, 2, i0:i0 + 128], qkT[:, 0, i0:i0 + 256],
                             start=True, stop=True)
            nc.scalar.activation(a0, ps, AF.Relu, bias=0.0, scale=1.0 / G)
            nc.vector.tensor_mul(a0[:, 0:128], a0[:, 0:128], triu)
            nc.vector.tensor_mul(a0, a0, a0)
            nc.vector.tensor_add(a0[:, 0:128], a0[:, 0:128], bdiag)
            nc.vector.tensor_add(a0[:, 128:256], a0[:, 128:256], bcorn)
            attn0.append(a0)

            a1 = p_a1.tile([128, 128], BF, tag="a1")
            ps = ps2.tile([128, 256], F32, tag="ps2")
            nc.tensor.matmul(ps[:, 0:128], qkT[:, 2, i0 + 128:i0 + 256],
                             qkT[:, 0, i0 + 128:i0 + 256], start=True, stop=True)
            nc.scalar.activation(a1, ps[:, 0:128], AF.Relu, bias=0.0, scale=1.0 / G)
            nc.vector.tensor_mul(a1, a1, triu)
            nc.vector.tensor_mul(a1, a1, a1)
            nc.vector.tensor_add(a1, a1, bdiag)
            attn1.append(a1)


        # ---- quad + conv boundary -> outT ----
        outT = p_outT.tile([128, ET, T], BF, tag="outT")
        for eh in range(2):
            e0 = eh * EH
            for g in range(NG):
                for et in range(8):
                    ec0 = e0 + et * 128
                    po = ps2.tile([128, 256], F32, tag="ps2", name="po")
                    nc.tensor.matmul(po, v_h[:, 2 * g, ec0:ec0 + 128], attn0[g],
                                     start=True, stop=False, skip_group_check=True)
                    nc.tensor.matmul(po[:, 128:256],
                                     v_h[:, 2 * g + 1, ec0:ec0 + 128],
                                     attn1[g], start=False, stop=False,
                                     skip_group_check=True)
                    nc.tensor.matmul(po[:, 0:32], tails[:, g, ec0:ec0 + 128], bprev,
                                     start=False, stop=True, skip_group_check=True)
                    nc.scalar.activation(outT[:, eh * 8 + et, g * G:(g + 1) * G],
                                         po, AF.Copy, bias=0.0, scale=1.0)

        # gate weights (DMA while PE chews on quad)
        wgb = p_w2.tile([128, KB, HID], BF, tag="w2", name="wgb")
        for kt in range(KB):
            nc.scalar.dma_start(wgb[:, kt, :],
                                ap["wgb"][kt * 128:(kt + 1) * 128, :])
        wg8 = p_so.tile([128, KG8, HID], F8, tag="so", name="wg8")
        nc.scalar.dma_start(wg8, ap["wg8"].rearrange("(kt p) e -> p kt e", p=128))

        # scheduler fence: without it the scheduler hoists the lin matmuls
        # (which wait on the AllGather+wsum) ahead of quad and parks the PE
        tc.no_sync_barrier()

        # ---- lin joined via DVE add into outT ----
        for eh in range(2):
            for g in range(NG):
                for et in range(eh * 8, eh * 8 + 8):
                    po = ps2.tile([128, 256], F32, tag="ps2", name="po")
                    nc.tensor.matmul(po, S_full[:, g, et * 128:(et + 1) * 128],
                                     qkT[:, 1, g * G:(g + 1) * G],
                                     start=True, stop=True)
                    nc.vector.tensor_add(outT[:, et, g * G:(g + 1) * G],
                                         outT[:, et, g * G:(g + 1) * G], po)

        # out-projection weights (DMA during lin/first gate)
        woutb = p_w1.tile([128, ET - KY8, DIM], BF, tag="w1", name="woutb")
        for kt in range(ET - KY8):
            nc.scalar.dma_start(woutb[:, kt, :],
                                ap["woutb"][kt * 128:(kt + 1) * 128, :])
        wout8 = p_lk.tile([128, KY8, DIM], F8, tag="lk", name="wout8")
        nc.scalar.dma_start(wout8, ap["wout8"].rearrange("(kt p) n -> p kt n",
                                                         p=128))

        if DEBUG_DUMPS:
            nc.sync.dma_start(ap["dbg_qkT"], qkT)
            nc.sync.dma_start(ap["dbg_vh"], v_h)
            nc.sync.dma_start(ap["dbg_outT"], outT)
            nc.sync.dma_start(ap["dbg_sfull"], S_full)

        # bf16 gate inputs (k-tiles 2..7), loaded during lin / earlier streams
        xtb = {}

        def load_xtb(s):
            halves = []
            for q in range(2):
                h = p_xtb.tile([128, KB // 2, T], BF, tag="xtb", name=f"xtb{s}_{q}")
                nc.sync.dma_start(
                    h, ap["xtb"][s, q * 384:(q + 1) * 384, :].rearrange(
                        "(kt p) t -> p kt t", p=128))
                halves.append(h)
            xtb[s] = halves

        load_xtb(0)
        load_xtb(1)

        # ---- gate + y per stream (th-paired gate, nch-paired y) ----
        for s in range(4):
            og8 = p_sf.tile([128, KY8, T], F8, tag="sf", name="og8")
            ogb = p_big.tile([128, ET - KY8, T], BF, tag="big", name="ogb")
            for et in range(ET):
                pg = [ps1.tile([128, 512], F32, tag="ps1", name="pg")
                      for _ in range(2)]
                for th in range(2):
                    nc.tensor.matmul(
                        pg[th], wg8[:, 0:KG8, et * 128:(et + 1) * 128],
                        x8f[s][:, 0:KG8, th * 512:(th + 1) * 512],
                        start=True, stop=False,
                        perf_mode=DRM, skip_group_check=True)
                for kt in range(KB):
                    xs_t = xtb[s][kt // 3][:, kt % 3, :]
                    for th in range(2):
                        nc.tensor.matmul(
                            pg[th], wgb[:, kt, et * 128:(et + 1) * 128],
                            xs_t[:, th * 512:(th + 1) * 512],
                            start=False, stop=(kt == KB - 1),
                            skip_group_check=True)
                for th in range(2):
                    t0 = th * 512
                    if et < KY8:
                        gt = p_qog.tile([128, 512], BF, tag="qog", name="gt")
                        nc.scalar.activation(gt, pg[th], AF.Silu,
                                             bias=bgate[:, et:et + 1], scale=1.0)
                        nc.vector.scalar_tensor_tensor(
                            og8[:, et, t0:t0 + 512], gt, XS,
                            outT[:, et, t0:t0 + 512], op0=ALU.mult, op1=ALU.mult)
                    else:
                        eb = et - KY8
                        nc.scalar.activation(ogb[:, eb, t0:t0 + 512], pg[th],
                                             AF.Silu, bias=bgate[:, et:et + 1],
                                             scale=1.0)
                        nc.vector.tensor_mul(ogb[:, eb, t0:t0 + 512],
                                             ogb[:, eb, t0:t0 + 512],
                                             outT[:, et, t0:t0 + 512])
            if s < 2:
                load_xtb(s + 2)
            if s == 0:
                load_x8f(1)
            elif s == 1:
                load_x8f(2)
            elif s == 2:
                load_x8f(3)
            for tl in range(TT):
                tsl = slice(tl * 128, (tl + 1) * 128)
                pn = [ps1.tile([128, 512], F32, tag="ps1", name="pn")
                      for _ in range(2)]
                for kp in range(KY8 // 2):
                    for nch in range(2):
                        nc.tensor.matmul(
                            pn[nch], og8[:, 2 * kp:2 * kp + 2, tsl],
                            wout8[:, 2 * kp:2 * kp + 2,
                                  nch * 512:(nch + 1) * 512],
                            start=(kp == 0), stop=False,
                            perf_mode=DRM, skip_group_check=True)
                for kt in range(ET - KY8):
                    for nch in range(2):
                        nc.tensor.matmul(
                            pn[nch], ogb[:, kt, tsl],
                            woutb[:, kt, nch * 512:(nch + 1) * 512],
                            start=False,
                            stop=(kt == ET - KY8 - 1 and not WITH_OBIAS),
                            skip_group_check=True)
                if WITH_OBIAS:
                    for nch in range(2):
                        nc.tensor.matmul(pn[nch], ones_t[0:1, 0:128],
                                         bout[0:1, nch * 512:(nch + 1) * 512],
                                         start=False, stop=True,
                                         skip_group_check=True)
                for nch in range(2):
                    n0 = nch * 512
                    xr = p_sob.tile([128, 512], F32, tag="xr", name="xr")
                    nc.sync.dma_start(xr, ap["xtok"][s, tsl, n0:n0 + 512])
                    ysb = p_tails.tile([128, 512], F32, tag="ysb", name="ysb",
                                       bufs=2)
                    nc.vector.scalar_tensor_tensor(
                        ysb, pn[nch], 0.0, xr, op0=ALU.add, op1=ALU.add)
                    nc.sync.dma_start(ap["y"][s, tsl, n0:n0 + 512], ysb)


def build_nc(with_vbias=None, with_obias=None):
    global WITH_VBIAS, WITH_OBIAS
    if with_vbias is not None:
        WITH_VBIAS = with_vbias
    if with_obias is not None:
        WITH_OBIAS = with_obias
    nc = bacc.Bacc("TRN2", target_bir_lowering=False, debug=False, num_devices=NC)
    ap = {}

    def dram(name, shape, dt, kind=None, addr_space=None):
        kw = {}
        if kind:
            kw["kind"] = kind
        if addr_space:
            kw["addr_space"] = addr_space
        ap[name] = nc.dram_tensor(name, shape, dt, **kw).ap()

    dram("xtb", [4, KB * 128, T], BF, kind="ExternalInput")
    dram("xt8", [3, DIM, T], F8, kind="ExternalInput")
    dram("xt08", [DIM, T], F8, kind="ExternalInput")
    dram("xh8", [DIM, 32], F8, kind="ExternalInput")
    dram("xtok", [4, T, DIM], F32, kind="ExternalInput")
    dram("wv8", [DIM, HID], F8, kind="ExternalInput")
    dram("wgb", [KB * 128, HID], BF, kind="ExternalInput")
    dram("wg8", [KG8 * 128, HID], F8, kind="ExternalInput")
    dram("wqk8", [DIM, DQK], F8, kind="ExternalInput")
    dram("woutb", [(ET - KY8) * 128, DIM], BF, kind="ExternalInput")
    dram("wout8", [KY8 * 128, DIM], F8, kind="ExternalInput")
    dram("wvb", [1, HID], BF, kind="ExternalInput")
    dram("bout", [1, DIM], BF, kind="ExternalInput")
    dram("bgate", [128, ET], F32, kind="ExternalInput")
    dram("bqk", [128, 1], F32, kind="ExternalInput")
    dram("triu", [128, 128], BF, kind="ExternalInput")
    dram("bdiag", [128, 128], BF, kind="ExternalInput")
    dram("bcorn", [128, 128], BF, kind="ExternalInput")
    dram("bprev", [32, 32], BF, kind="ExternalInput")
    dram("hmask", [32, 1], F32, kind="ExternalInput")
    dram("wsumw", [128, NC], F32, kind="ExternalInput")
    if DEBUG_DUMPS:
        dram("dbg_qkT", [128, 4, T], BF, kind="ExternalOutput")
        dram("dbg_vh", [128, TT, HID], F8, kind="ExternalOutput")
        dram("dbg_outT", [128, ET, T], BF, kind="ExternalOutput")
        dram("dbg_sfull", [128, NG, HID], BF, kind="ExternalOutput")
    dram("cc_warm_in", [128, 16], BF)
    dram("cc_warm_out", [NC * 128, 16], BF, addr_space="Shared")
    dram("cc_in0", [128, EH], BF)
    dram("cc_out0", [NC * 128, EH], BF, addr_space="Shared")
    dram("cc_in1", [128, EH], BF)
    dram("cc_out1", [NC * 128, EH], BF, addr_space="Shared")
    dram("y", [4, T, DIM], F32, kind="ExternalOutput")

    with tile.TileContext(nc) as tc:
        _emit(tc, ap)
    nc.compile()
    return nc


def _f8(a):
    return np.clip(a, -240.0, 240.0).astype(fp8)


def host_prep(inputs):
    """Pure layout transforms: shard, transpose, cast, build conv-band consts."""
    x = np.ascontiguousarray(np.asarray(inputs["x"], np.float32)[0])  # [4, N, DIM]
    W_h = np.asarray(inputs["W_h"], np.float32)
    b_h = np.asarray(inputs["b_h"], np.float32)
    W_qk = np.asarray(inputs["W_qk"], np.float32)
    b_qk = np.asarray(inputs["b_qk"], np.float32)
    W_out = np.asarray(inputs["W_out"], np.float32)
    b_out = np.asarray(inputs["b_out"], np.float32)
    cw = np.asarray(inputs["conv_w"], np.float32)

    jj = np.arange(128)[:, None]
    ii = np.arange(128)[None, :]
    d = ii - jj
    triu = (ii >= jj).astype(bf16)
    bdiag = np.where((d >= 0) & (d <= 31), cw[np.clip(31 - d, 0, 62)], 0.0).astype(bf16)
    dc = (ii + 128) - jj
    bcorn = np.where((dc >= 0) & (dc <= 31),
                     cw[np.clip(31 - dc, 0, 62)], 0.0).astype(bf16)
    jt = np.arange(32)[:, None]
    ip = np.arange(32)[None, :]
    dp = ip + 32 - jt
    bprev = np.where((dp >= 1) & (dp <= 31),
                     cw[np.clip(31 - dp, 0, 62)], 0.0).astype(bf16)

    kb0 = KG8 * 128
    ky0 = KY8 * 128
    common = {
        "wv8": _f8(W_h[:, :HID] * WS),
        "wgb": np.ascontiguousarray(W_h[kb0:, HID:]).astype(bf16),
        "wg8": _f8(W_h[:kb0, HID:] * WS),
        "wqk8": _f8(W_qk * WS),
        "woutb": np.ascontiguousarray(W_out[ky0:, :]).astype(bf16),
        "wout8": _f8(W_out[:ky0, :] * WS),
        "wvb": b_h[None, :HID].astype(bf16),
        "bout": b_out[None, :].astype(bf16),
        "bgate": np.ascontiguousarray(b_h[HID:].reshape(ET, 128).T).astype(np.float32),
        "bqk": b_qk[:, None].astype(np.float32),
        "triu": triu, "bdiag": bdiag, "bcorn": bcorn, "bprev": bprev,
    }

    in_maps = []
    for c in range(NC):
        sl = slice(c * T, (c + 1) * T)
        x_c = x[:, sl, :]
        xtb = np.zeros((4, KB * 128, T), bf16)
        xt8 = np.zeros((3, DIM, T), fp8)
        for s in range(4):
            xT = x_c[s].T
            xtb[s] = xT[kb0:].astype(bf16)
            if s > 0:
                xt8[s - 1] = _f8(xT * XS)
        xt08 = _f8(x_c[0].T * XS)
        if c > 0:
            xh8 = _f8(np.ascontiguousarray(x[0, c * T - 32:c * T, :].T) * XS)
        else:
            xh8 = np.zeros((DIM, 32), fp8)
        m = dict(common)
        m["xtb"] = xtb
        m["xt8"] = xt8
        m["xt08"] = xt08
        m["xh8"] = xh8
        m["xtok"] = np.ascontiguousarray(x_c)
        m["hmask"] = np.full((32, 1), 1.0 if c > 0 else 0.0, np.float32)
        w = np.zeros((128, NC), np.float32)
        w[:, :c] = 1.0
        m["wsumw"] = w
        in_maps.append(m)
    return in_maps


_NC_PROG = None
_NC_FLAGS = None


def kernel(**inputs):
    global _NC_PROG, _NC_FLAGS
    b_h = np.asarray(inputs["b_h"], np.float32)
    b_out = np.asarray(inputs["b_out"], np.float32)
    flags = (bool(np.any(b_h[:HID])), bool(np.any(b_out)))
    if _NC_PROG is None or _NC_FLAGS != flags:
        _NC_PROG = build_nc(with_vbias=flags[0], with_obias=flags[1])
        _NC_FLAGS = flags
    in_maps = host_prep(inputs)
    res = run_bass_kernel_spmd(_NC_PROG, in_maps, list(range(NC)))
    y = np.stack([res.results[c]["y"] for c in range(NC)], axis=1)  # [4, NC, T, DIM]
    return np.ascontiguousarray(y.reshape(4, NSEQ, DIM)[None]).astype(np.float32)
